# revision 35
# baseline (speedup 1.0000x reference)
"""Trainium2 Bass kernel for nn_DiTBlock (HGRN-attention DiT block).

Sharding: 8 cores = 4 batches x 2 half-sequences (1024 tokens each).
All matmuls are exact-integer bf16 matmuls (activations quantized to
int8-range integers in bf16; ternary weights quantized on host).

v6 structure:
  - per-token-tile pipelining (LN/quant/round per 128-token tile)
  - activation transposes via DMA-XBAR (bf16); only the f32 h-transpose
    uses the PE
  - AllGathers overlapped (adaln AG under the LN-stats sweep, the scan
    carry AG under interleaved wg matmuls)
  - exact fp32 C_MAGIC rounding; subtract step on ScalarE
  - scan outputs (ha/cam) spilled to DRAM per chunk, reloaded for the
    carry fix; h2 spilled pre-round as bf16
  - weight loads on the scalar/gpsimd HWDGE queues so they never queue
    behind compute-dependent DMAs
"""
import functools
import numpy as np
import ml_dtypes

import concourse.bass as bass
import concourse.bacc as bacc_mod
import concourse.mybir as mybir
import concourse.tile as tile
from concourse.masks import make_identity
from concourse.bass_utils import run_bass_kernel_spmd

BF16 = ml_dtypes.bfloat16
F32 = mybir.dt.float32
BF = mybir.dt.bfloat16
U32 = mybir.dt.uint32
AL = mybir.AluOpType
AF = mybir.ActivationFunctionType
AX = mybir.AxisListType

B, T, D = 4, 2048, 1024
TOK = 1024          # tokens per core
NH, HD = 16, 64
MLP = 4096
N_CORES = 8
CMAGIC = float(1.5 * 2 ** 23)
MAGIC_U32 = 0x5F3759DF


def _quant_w(w):
    invws = float(np.clip(np.abs(w).mean(dtype=np.float64), 1e-5, None))
    m = np.clip(np.round(w.astype(np.float64) / invws), -1, 1).astype(np.float32)
    return np.ascontiguousarray(m.astype(BF16)), np.float32(invws)


def _build(iw):
    nc = bacc_mod.Bacc("TRN2", target_bir_lowering=False)

    x_sl = nc.declare_dram_parameter("x_sl", [TOK, D], F32, isOutput=False)
    c_cols = nc.declare_dram_parameter("c_cols", [128, 8, B], F32, isOutput=False)
    adw_sl = nc.declare_dram_parameter("adw_sl", [D, 768], F32, isOutput=False)
    adb_sl = nc.declare_dram_parameter("adb_sl", [1, 768], F32, isOutput=False)
    mask8 = nc.declare_dram_parameter("mask8", [N_CORES, 1], F32, isOutput=False)
    bsel = nc.declare_dram_parameter("bsel", [1, B], F32, isOutput=False)
    bmask = nc.declare_dram_parameter("bmask", [B, 1], F32, isOutput=False)
    gnr = nc.declare_dram_parameter("gnr", [1, D], F32, isOutput=False)
    wiT = nc.declare_dram_parameter("wiT", [D, D], BF, isOutput=False)
    wfT = nc.declare_dram_parameter("wfT", [D, D], BF, isOutput=False)
    wgT = nc.declare_dram_parameter("wgT", [D, D], BF, isOutput=False)
    woT = nc.declare_dram_parameter("woT", [D, D], BF, isOutput=False)
    gwT = nc.declare_dram_parameter("gwT", [D, 2 * MLP], BF, isOutput=False)
    dwT = nc.declare_dram_parameter("dwT", [MLP, D], BF, isOutput=False)
    out_sl = nc.declare_dram_parameter("out_sl", [TOK, D], F32, isOutput=True)

    cc1_in = nc.dram_tensor("cc1_in", [B, 768], F32)
    cc1_out = nc.dram_tensor("cc1_out", [N_CORES * B, 768], F32,
                             addr_space="Shared")
    cc2_in = nc.dram_tensor("cc2_in", [D], F32)
    cc2_out = nc.dram_tensor("cc2_out", [N_CORES, D], F32, addr_space="Shared")

    RG = [list(range(N_CORES))]

    with tile.TileContext(nc) as tc:
        import contextlib
        es = contextlib.ExitStack()
        with es:
            cst = es.enter_context(tc.tile_pool(name="cst", bufs=1))
            ps = es.enter_context(tc.tile_pool(name="ps", bufs=4, space="PSUM"))
            psm = es.enter_context(tc.tile_pool(name="psm", bufs=1, space="PSUM"))
            pst = es.enter_context(tc.tile_pool(name="pst", bufs=2, space="PSUM"))
            dr = es.enter_context(tc.tile_pool(name="dr", bufs=1, space="DRAM"))

            def pmm():
                return ps.tile([128, 512], F32, tag="mm", name="mm")

            def newton_rsqrt(sb, x_ap, scale, bias, shape, tag, iters=3):
                """rsqrt(x*scale + bias) via bit-magic + Newton iters."""
                t = sb.tile(shape, F32, tag=tag + "_t", name=tag + "_t")
                nc.vector.tensor_scalar(out=t, in0=x_ap, scalar1=float(scale),
                                        scalar2=float(bias), op0=AL.mult,
                                        op1=AL.add)
                y = sb.tile(shape, F32, tag=tag + "_y", name=tag + "_y")
                sh = sb.tile(shape, F32, tag=tag + "_s", name=tag + "_s")
                nc.vector.tensor_scalar(out=sh[:].bitcast(U32),
                                        in0=t[:].bitcast(U32), scalar1=1,
                                        scalar2=None,
                                        op0=AL.logical_shift_right)
                mg = sb.tile(shape, F32, tag=tag + "_m", name=tag + "_m")
                nc.vector.memset(mg[:].bitcast(U32), MAGIC_U32)
                nc.vector.tensor_tensor(out=y[:].bitcast(U32),
                                        in0=mg[:].bitcast(U32),
                                        in1=sh[:].bitcast(U32), op=AL.subtract)
                e = sb.tile(shape, F32, tag=tag + "_e", name=tag + "_e")
                for _ in range(iters):
                    nc.vector.tensor_tensor(out=e, in0=y, in1=y, op=AL.mult)
                    nc.vector.tensor_tensor(out=e, in0=e, in1=t, op=AL.mult)
                    nc.vector.tensor_scalar(out=e, in0=e, scalar1=-0.5,
                                            scalar2=1.5, op0=AL.mult,
                                            op1=AL.add)
                    nc.vector.tensor_tensor(out=y, in0=y, in1=e, op=AL.mult)
                return y

            def quant_cols(sb, am_col, ss_col, dk, q_col, dq_col, dq_scale,
                           shape, tag):
                """q = 127/max(am,1e-5); dq = am*rsqrt(ss/dk+1e-8)*scale/127"""
                amc = sb.tile(shape, F32, tag=tag + "amc", name=tag + "amc")
                nc.vector.tensor_scalar(out=amc, in0=am_col, scalar1=1e-5,
                                        scalar2=None, op0=AL.max)
                rec = sb.tile(shape, F32, tag=tag + "rec", name=tag + "rec")
                nc.vector.reciprocal(out=rec, in_=amc)
                nc.vector.tensor_scalar(out=q_col, in0=rec, scalar1=127.0,
                                        scalar2=None, op0=AL.mult)
                rs = newton_rsqrt(sb, ss_col, 1.0 / dk, 1e-8, shape,
                                  tag + "rs", iters=2)
                nc.vector.tensor_tensor(out=dq_col, in0=amc, in1=rs,
                                        op=AL.mult)
                nc.vector.tensor_scalar(out=dq_col, in0=dq_col,
                                        scalar1=float(dq_scale) / 127.0,
                                        scalar2=None, op0=AL.mult)

            # ---------------- consts (whole-kernel lifetime) -------------
            identf = cst.tile([128, 128], F32)
            make_identity(nc, identf)
            ones_row = cst.tile([1, 128], F32)
            nc.vector.memset(ones_row, 1.0)
            negmagic = cst.tile([128, 1], F32)
            nc.vector.memset(negmagic, -CMAGIC)
            mask_sb = cst.tile([N_CORES, 1], F32)
            nc.sync.dma_start(out=mask_sb, in_=mask8[:, :])
            bsel_sb = cst.tile([1, B], F32)
            nc.sync.dma_start(out=bsel_sb, in_=bsel[:, :])
            bmask_sb = cst.tile([B, 1], F32)
            nc.sync.dma_start(out=bmask_sb, in_=bmask[:, :])

            def round_bf16(sb, src, q_col, tagp, bufs=3):
                """round(src*q) -> integer-valued bf16 tile via fp32 magic."""
                t2 = sb.tile([128, src.free_size()], F32, tag=tagp + "t2",
                             name=tagp + "t2")
                nc.vector.tensor_scalar(out=t2, in0=src, scalar1=q_col,
                                        scalar2=CMAGIC, op0=AL.mult,
                                        op1=AL.add)
                kq = sb.tile([128, src.free_size()], BF, tag=tagp + "kq",
                             name=tagp + "kq", bufs=bufs)
                nc.scalar.activation(out=kq, in_=t2, func=AF.Identity,
                                     bias=negmagic)
                return kq

            q127A = cst.tile([128, 8], F32); dqA = cst.tile([128, 8], F32)
            dqAg = cst.tile([128, 8], F32)
            q127O = cst.tile([128, 8], F32); dqOo = cst.tile([128, 8], F32)
            q127C = cst.tile([128, 8], F32); dqCg = cst.tile([128, 8], F32)
            q127D = cst.tile([128, 8], F32); dqDo = cst.tile([128, 8], F32)
            B_gn = cst.tile([128, D], F32)
            B_g1 = cst.tile([128, D], F32)
            B_sh2 = cst.tile([128, D], F32)
            B_sc2 = cst.tile([128, D], F32)
            B_g2 = cst.tile([128, D], F32)

            params_d = dr.tile([6 * D], F32, tag="params")
            ha_d = dr.tile([TOK, D], F32, tag="had")
            cam_d = dr.tile([TOK, D], BF, tag="camd")
            h2_d = dr.tile([TOK, MLP], BF, tag="h2d")

            # whole-kernel SBUF: xqT (-> x2qT) and xnew
            big = es.enter_context(tc.tile_pool(name="big", bufs=1))
            xqT = big.tile([128, 8, TOK], BF, tag="xq", bufs=1)

            def bcast_row(row_ap, dst, plus1=False):
                for ch in range(0, D, 512):
                    pb = pst.tile([128, 512], F32, tag="aux", name="aux")
                    nc.tensor.matmul(pb, ones_row, row_ap[:, ch:ch + 512],
                                     start=True, stop=True)
                    if plus1:
                        nc.scalar.activation(out=dst[:, ch:ch + 512], in_=pb,
                                             func=AF.Identity, bias=1.0)
                    else:
                        nc.scalar.copy(out=dst[:, ch:ch + 512], in_=pb)

            # =========== phase 0: adaln (+AG) and LN stats ===========
            with tc.tile_pool(name="p0", bufs=2) as p0:
                B_sh1 = p0.tile([128, D], F32, tag="Bsh1", bufs=1)
                B_sc1 = p0.tile([128, D], F32, tag="Bsc1", bufs=1)
                gnr_sb = p0.tile([1, D], F32, tag="gnr", bufs=1)
                nc.sync.dma_start(out=gnr_sb, in_=gnr[:, :])

                # adaln: this core computes 768 of the 6144 outputs
                c_sb = p0.tile([128, 8, B], F32, tag="csb", bufs=1)
                nc.sync.dma_start(out=c_sb, in_=c_cols[:, :, :])
                cs_sb = p0.tile([128, 8, B], F32, tag="cssb", bufs=1)
                nc.scalar.activation(out=cs_sb, in_=c_sb, func=AF.Silu)
                adb_sb = p0.tile([1, 768], F32, tag="adb", bufs=1)
                nc.sync.dma_start(out=adb_sb, in_=adb_sl[:, :])
                psA = psm.tile([B, 512], F32, tag="sm")
                psB = psm.tile([B, 256], F32, tag="sm2")
                for j in range(8):
                    adw_j = p0.tile([128, 768], F32, tag="adw")
                    nc.sync.dma_start(out=adw_j,
                                      in_=adw_sl[128 * j:128 * (j + 1), :])
                    nc.tensor.matmul(psA, cs_sb[:, j, :], adw_j[:, 0:512],
                                     start=(j == 0), stop=False)
                    nc.tensor.matmul(psB, cs_sb[:, j, :], adw_j[:, 512:768],
                                     start=(j == 0), stop=False)
                nc.tensor.matmul(psA, bsel_sb, adb_sb[:, 0:512],
                                 start=False, stop=True)
                nc.tensor.matmul(psB, bsel_sb, adb_sb[:, 512:768],
                                 start=False, stop=True)
                ad_sb = p0.tile([B, 768], F32, tag="adsb", bufs=1)
                nc.scalar.copy(out=ad_sb[:, 0:512], in_=psA)
                nc.scalar.copy(out=ad_sb[:, 512:768], in_=psB)
                nc.sync.dma_start(out=cc1_in[:, :], in_=ad_sb)
                nc.gpsimd.collective_compute(
                    "AllGather", AL.bypass, ins=[cc1_in[:]],
                    outs=[cc1_out[:]], replica_groups=RG)

                # LN stats pass (params-independent -> overlaps the AG)
                amA = p0.tile([128, 8], F32, tag="amA", bufs=1)
                ssA = p0.tile([128, 8], F32, tag="ssA", bufs=1)
                rstdA = p0.tile([128, 8], F32, tag="rstdA", bufs=1)
                nmrA = p0.tile([128, 8], F32, tag="nmrA", bufs=1)
                for i in range(8):
                    xt = p0.tile([128, D], F32, tag="xt", bufs=2)
                    nc.sync.dma_start(out=xt,
                                      in_=x_sl[128 * i:128 * (i + 1), :])
                    st = p0.tile([128, 2, 6], F32, tag="bst")
                    xr = xt.rearrange("p (s d) -> p s d", s=2)
                    for s2 in range(2):
                        nc.vector.bn_stats(out=st[:, s2, :], in_=xr[:, s2, :])
                    mv = p0.tile([128, 2], F32, tag="bmv")
                    nc.vector.bn_aggr(out=mv, in_=st)
                    rstdLN = newton_rsqrt(p0, mv[:, 1:2], 1.0, 1e-6,
                                          [128, 1], "rLN")
                    nc.vector.tensor_copy(out=rstdA[:, i:i + 1], in_=rstdLN)
                    nc.vector.tensor_tensor(out=nmrA[:, i:i + 1],
                                            in0=mv[:, 0:1], in1=rstdLN,
                                            op=AL.mult)
                nc.vector.tensor_scalar(out=nmrA, in0=nmrA, scalar1=-1.0,
                                        scalar2=None, op0=AL.mult)

                # select this batch's row per 768-block, stage through DRAM
                for r in range(8):
                    ag_r = p0.tile([B, 768], F32, tag="ag1")
                    nc.sync.dma_start(out=ag_r,
                                      in_=cc1_out[4 * r:4 * (r + 1), :])
                    pp1 = psm.tile([1, 512], F32, tag="sm")
                    pp2 = psm.tile([1, 256], F32, tag="sm2")
                    nc.tensor.matmul(pp1, bmask_sb, ag_r[:, 0:512],
                                     start=True, stop=True)
                    nc.tensor.matmul(pp2, bmask_sb, ag_r[:, 512:768],
                                     start=True, stop=True)
                    rb7 = p0.tile([1, 768], F32, tag="rb7", bufs=1)
                    nc.scalar.copy(out=rb7[:, 0:512], in_=pp1)
                    nc.scalar.copy(out=rb7[:, 512:768], in_=pp2)
                    nc.sync.dma_start(
                        out=params_d[768 * r:768 * (r + 1)].rearrange(
                            "(one c) -> one c", one=1),
                        in_=rb7)
                B_list = [(B_sh1, False), (B_sc1, True), (B_g1, False),
                          (B_sh2, False), (B_sc2, True), (B_g2, False)]
                for k, (dst, plus1) in enumerate(B_list):
                    rbk = p0.tile([1, D], F32, tag="rbk", bufs=1)
                    nc.sync.dma_start(
                        out=rbk,
                        in_=params_d[D * k:D * (k + 1)].rearrange(
                            "(one c) -> one c", one=1))
                    bcast_row(rbk, dst, plus1=plus1)
                bcast_row(gnr_sb, B_gn)

                # ===== phase 1: modulate + quant + round (per tile) =====
                for i in range(8):
                    xt = p0.tile([128, D], F32, tag="xt2", bufs=2)
                    nc.sync.dma_start(out=xt,
                                      in_=x_sl[128 * i:128 * (i + 1), :])
                    u = p0.tile([128, D], F32, tag="u")
                    nc.scalar.activation(out=u, in_=xt, func=AF.Identity,
                                         scale=rstdA[:, i:i + 1],
                                         bias=nmrA[:, i:i + 1])
                    ttm = p0.tile([128, D], F32, tag="ttm")
                    nc.vector.tensor_tensor(out=ttm, in0=u, in1=B_sc1,
                                            op=AL.mult)
                    moda = p0.tile([128, D], F32, tag="moda", bufs=2)
                    nc.vector.tensor_tensor(out=moda, in0=ttm, in1=B_sh1,
                                            op=AL.add)
                    nc.vector.tensor_reduce(out=amA[:, i:i + 1], in_=moda,
                                            axis=AX.X, op=AL.max,
                                            apply_absolute_value=True)
                    sqs = p0.tile([128, D], BF, tag="sqs")
                    nc.scalar.activation(out=sqs, in_=moda, func=AF.Square,
                                         accum_out=ssA[:, i:i + 1])
                    quant_cols(p0, amA[:, i:i + 1], ssA[:, i:i + 1], D,
                               q127A[:, i:i + 1], dqA[:, i:i + 1], 1.0,
                               [128, 1], "qa")
                    nc.vector.tensor_scalar(out=dqAg[:, i:i + 1],
                                            in0=dqA[:, i:i + 1],
                                            scalar1=float(iw["g"]),
                                            scalar2=None, op0=AL.mult)
                    kq = round_bf16(p0, moda, q127A[:, i:i + 1], "ra")
                    nc.sync.dma_start(out=xqT[:, :, 128 * i:128 * (i + 1)],
                                      in_=kq, transpose=True)

            # ====== phase 2: i/f + g matmuls + scan (chunked) ======
            pG = tc.tile_pool(name="pG", bufs=1)
            pgs = pG.__enter__()
            gs = pgs.tile([128, 8, D], F32, tag="gs", bufs=1)
            with tc.tile_pool(name="p2", bufs=2) as p2:
                # Sb_i / Sb_f: dq row broadcast over partitions, scaled by iw
                Sb_i = p2.tile([128, TOK], F32, tag="Sbi", bufs=1)
                Sb_f = p2.tile([128, TOK], F32, tag="Sbf", bufs=1)
                dqrow_sb = p2.tile([1, D], F32, tag="dqrow", bufs=1)
                for i8 in range(8):
                    ptr = psm.tile([1, 128], F32, tag="sm")
                    nc.tensor.transpose(ptr, dqA[:, i8:i8 + 1], identf)
                    nc.scalar.copy(out=dqrow_sb[:, 128 * i8:128 * (i8 + 1)],
                                   in_=ptr)
                oi = p2.tile([1, 128], F32, tag="oi", bufs=1)
                nc.vector.memset(oi, float(iw["i"]))
                of = p2.tile([1, 128], F32, tag="of", bufs=1)
                nc.vector.memset(of, float(iw["f"]))
                for ch in range(0, TOK, 512):
                    pb = pst.tile([128, 512], F32, tag="aux", name="aux")
                    nc.tensor.matmul(pb, oi, dqrow_sb[:, ch:ch + 512],
                                     start=True, stop=True)
                    nc.scalar.copy(out=Sb_i[:, ch:ch + 512], in_=pb)
                    pb2 = pst.tile([128, 512], F32, tag="aux", name="aux")
                    nc.tensor.matmul(pb2, of, dqrow_sb[:, ch:ch + 512],
                                     start=True, stop=True)
                    nc.scalar.copy(out=Sb_f[:, ch:ch + 512], in_=pb2)

                wf_sb = p2.tile([128, 8, D], BF, tag="wfsb", bufs=1)
                nc.scalar.dma_start(
                    out=wf_sb,
                    in_=wfT[:, :].rearrange("(a p) q -> p a q", p=128))
                wi_sb = p2.tile([128, 8, D], BF, tag="wisb", bufs=1)
                nc.scalar.dma_start(
                    out=wi_sb,
                    in_=wiT[:, :].rearrange("(a p) q -> p a q", p=128))
                wg_sb = p2.tile([128, 8, D], BF, tag="wgsb", bufs=1)
                nc.scalar.dma_start(
                    out=wg_sb,
                    in_=wgT[:, :].rearrange("(a p) q -> p a q", p=128))

                def g_mms(trange):
                    for t in trange:
                        for ck2 in range(0, D, 512):
                            pg = pmm()
                            for j in range(8):
                                nc.tensor.matmul(
                                    pg, xqT[:, j, 128 * t:128 * (t + 1)],
                                    wg_sb[:, j, ck2:ck2 + 512],
                                    start=(j == 0), stop=(j == 7))
                            scr = p2.tile([128, 512], F32, tag="gscr",
                                          bufs=2)
                            nc.scalar.activation(out=scr, in_=pg,
                                                 func=AF.Silu,
                                                 scale=dqAg[:, t:t + 1])
                            nc.vector.tensor_tensor(
                                out=gs[:, t, ck2:ck2 + 512], in0=scr,
                                in1=B_gn[:, ck2:ck2 + 512], op=AL.mult)

                ha_last = p2.tile([128, 8], F32, tag="halast", bufs=1)
                cam_last = p2.tile([128, 8], F32, tag="camlast", bufs=1)

                def scan_chain(ft, it, m, ck, cki):
                    sigf = p2.tile([128, 512], F32, tag="sigf", bufs=2)
                    nc.scalar.activation(out=sigf, in_=ft, func=AF.Sigmoid)
                    omf = p2.tile([128, 512], F32, tag="omf", bufs=2)
                    nc.scalar.activation(out=omf, in_=ft, func=AF.Sigmoid,
                                         scale=-1.0)
                    sili = p2.tile([128, 512], F32, tag="sili", bufs=2)
                    nc.scalar.activation(out=sili, in_=it, func=AF.Silu)
                    ifin = p2.tile([128, 512], F32, tag="ifin", bufs=2)
                    nc.vector.tensor_tensor(out=ifin, in0=sili, in1=omf,
                                            op=AL.mult)
                    ha_c = p2.tile([128, 512], F32, tag="hac", bufs=3)
                    init_h = 0.0 if cki == 0 else ha_last[:, m:m + 1]
                    nc.vector.tensor_tensor_scan(
                        ha_c, sigf, ifin, init_h, op0=AL.mult, op1=AL.add)
                    cam_c = p2.tile([128, 512], BF, tag="camc", bufs=3)
                    init_c = 1.0 if cki == 0 else cam_last[:, m:m + 1]
                    nc.vector.tensor_tensor_scan(
                        cam_c, sigf, sigf, init_c, op0=AL.mult, op1=AL.bypass)
                    nc.sync.dma_start(
                        out=ha_d[128 * m:128 * (m + 1), ck:ck + 512],
                        in_=ha_c)
                    nc.sync.dma_start(
                        out=cam_d[128 * m:128 * (m + 1), ck:ck + 512],
                        in_=cam_c)
                    if cki == 0:
                        nc.vector.tensor_copy(out=ha_last[:, m:m + 1],
                                              in_=ha_c[:, 511:512])
                        nc.vector.tensor_copy(out=cam_last[:, m:m + 1],
                                              in_=cam_c[:, 511:512])
                    else:
                        nc.sync.dma_start(
                            out=cc2_in[128 * m:128 * (m + 1)].rearrange(
                                "(p one) -> p one", one=1),
                            in_=ha_c[:, 511:512])

                # software-pipelined: psum evacuations lead the scan chain
                # by one iteration in the DVE stream so the PE never
                # starves on psum slots
                pending = None
                for cki, ck in enumerate(range(0, TOK, 512)):
                    for m in range(8):
                        pf = pmm()
                        pi = pmm()
                        for j in range(8):
                            nc.tensor.matmul(
                                pf, wf_sb[:, j, 128 * m:128 * (m + 1)],
                                xqT[:, j, ck:ck + 512],
                                start=(j == 0), stop=(j == 7))
                        for j in range(8):
                            nc.tensor.matmul(
                                pi, wi_sb[:, j, 128 * m:128 * (m + 1)],
                                xqT[:, j, ck:ck + 512],
                                start=(j == 0), stop=(j == 7))
                        ft = p2.tile([128, 512], F32, tag="ftm", bufs=2)
                        nc.vector.tensor_tensor(out=ft, in0=pf,
                                                in1=Sb_f[:, ck:ck + 512],
                                                op=AL.mult)
                        it = p2.tile([128, 512], F32, tag="itm", bufs=2)
                        nc.vector.tensor_tensor(out=it, in0=pi,
                                                in1=Sb_i[:, ck:ck + 512],
                                                op=AL.mult)
                        if pending is not None:
                            scan_chain(*pending)
                        pending = (ft, it, m, ck, cki)
                    if cki == 1:
                        scan_chain(*pending)
                        pending = None
                    g_mms(range(0, 4) if cki == 0 else range(4, 8))
                nc.gpsimd.collective_compute(
                    "AllGather", AL.bypass, ins=[cc2_in[:]], outs=[cc2_out[:]],
                    replica_groups=RG)

            # =========== phase 3: carry fix + hT (PE transpose) ==========
            pO = tc.tile_pool(name="pO", bufs=1)
            pos_ = pO.__enter__()
            oqT = pos_.tile([128, 8, D], BF, tag="oqT", bufs=1)
            wo_sb = pos_.tile([128, 8, D], BF, tag="wosb", bufs=1)
            nc.scalar.dma_start(
                out=wo_sb,
                in_=woT[:, :].rearrange("(a p) q -> p a q", p=128))
            pH3 = tc.tile_pool(name="pH3", bufs=1)
            ph3_ = pH3.__enter__()
            hT = ph3_.tile([128, 8, D], F32, tag="hT", bufs=1)
            with tc.tile_pool(name="p3", bufs=2) as p3:
                ag2 = p3.tile([N_CORES, D], F32, tag="ag2", bufs=1)
                nc.sync.dma_start(out=ag2, in_=cc2_out[:, :])
                for m in range(8):
                    pc = psm.tile([128, 1], F32, tag="sm")
                    nc.tensor.matmul(pc, ag2[:, 128 * m:128 * (m + 1)],
                                     mask_sb, start=True, stop=True)
                    carry = p3.tile([128, 1], F32, tag="carry")
                    nc.scalar.copy(out=carry, in_=pc)
                    har = p3.tile([128, TOK], F32, tag="har", bufs=2)
                    nc.sync.dma_start(out=har,
                                      in_=ha_d[128 * m:128 * (m + 1), :])
                    camr = p3.tile([128, TOK], BF, tag="camr", bufs=2)
                    nc.sync.dma_start(out=camr,
                                      in_=cam_d[128 * m:128 * (m + 1), :])
                    hfix = p3.tile([128, TOK], F32, tag="hfix", bufs=2)
                    nc.vector.scalar_tensor_tensor(out=hfix, in0=camr,
                                                   scalar=carry, in1=har,
                                                   op0=AL.mult, op1=AL.add)
                    for g4 in range(0, 8, 4):
                        tp = pst.tile([128, 512], F32, tag="aux")
                        for jj in range(4):
                            t_i = g4 + jj
                            nc.tensor.transpose(
                                tp[:, 128 * jj:128 * (jj + 1)],
                                hfix[:, 128 * t_i:128 * (t_i + 1)], identf)
                        for jj in range(4):
                            t_i = g4 + jj
                            if jj % 2 == 0:
                                nc.scalar.copy(
                                    out=hT[:, t_i, 128 * m:128 * (m + 1)],
                                    in_=tp[:, 128 * jj:128 * (jj + 1)])
                            else:
                                nc.vector.tensor_copy(
                                    out=hT[:, t_i, 128 * m:128 * (m + 1)],
                                    in_=tp[:, 128 * jj:128 * (jj + 1)])

            # =========== phase 4: gnorm-swish-gate + o quant ===========
            with tc.tile_pool(name="p4", bufs=2) as p4:
                amO = p4.tile([128, 8], F32, tag="amO", bufs=1)
                ssO = p4.tile([128, 8], F32, tag="ssO", bufs=1)
                for t in range(8):
                    sq = p4.tile([128, D], F32, tag="sq")
                    nc.scalar.activation(out=sq, in_=hT[:, t, :],
                                         func=AF.Square)
                    msh = p4.tile([128, 16], F32, tag="msh")
                    nc.vector.tensor_reduce(
                        out=msh,
                        in_=sq.rearrange("p (h d) -> p h d", h=NH),
                        axis=AX.X, op=AL.add)
                    rstdH = newton_rsqrt(p4, msh, 1.0 / HD, 1e-5, [128, 16],
                                         "rH")
                    hn = p4.tile([128, D], F32, tag="hn")
                    rb = bass.AP(tensor=rstdH.tensor, offset=rstdH.offset,
                                 ap=[rstdH.ap[0], [1, NH], [0, HD]])
                    nc.vector.tensor_tensor(
                        out=hn.rearrange("p (h d) -> p h d", h=NH),
                        in0=hT[:, t, :].rearrange("p (h d) -> p h d", h=NH),
                        in1=rb, op=AL.mult)
                    oa = p4.tile([128, D], F32, tag="oa", bufs=2)
                    nc.vector.tensor_tensor(out=oa, in0=hn, in1=gs[:, t, :],
                                            op=AL.mult)
                    nc.vector.tensor_reduce(out=amO[:, t:t + 1], in_=oa,
                                            axis=AX.X, op=AL.max,
                                            apply_absolute_value=True)
                    sqo = p4.tile([128, D], BF, tag="sqo", bufs=1)
                    nc.scalar.activation(out=sqo, in_=oa, func=AF.Square,
                                         accum_out=ssO[:, t:t + 1])
                    quant_cols(p4, amO[:, t:t + 1], ssO[:, t:t + 1], D,
                               q127O[:, t:t + 1], dqOo[:, t:t + 1], iw["o"],
                               [128, 1], "qo")
                    kq = round_bf16(p4, oa, q127O[:, t:t + 1], "ro")
                    nc.sync.dma_start(out=oqT[:, :, 128 * t:128 * (t + 1)],
                                      in_=kq, transpose=True)
            pH3.__exit__(None, None, None)  # free hT

            # ====== phase 5: wo matmul + residual + LN2 + quant ======
            xnew = big.tile([128, 8, D], F32, tag="xnew", bufs=1)
            x2qT = big.tile([128, 8, TOK], BF, tag="xq", bufs=1)
            with tc.tile_pool(name="p5", bufs=2) as p5:
                amC = p5.tile([128, 8], F32, tag="amC", bufs=1)
                ssC = p5.tile([128, 8], F32, tag="ssC", bufs=1)
                for t in range(8):
                    xr2 = p5.tile([128, D], F32, tag="xr2", bufs=2)
                    nc.sync.dma_start(out=xr2,
                                      in_=x_sl[128 * t:128 * (t + 1), :])
                    xn = xnew[:, t, :]
                    for ck in range(0, D, 512):
                        pw = pmm()
                        for j in range(8):
                            nc.tensor.matmul(
                                pw, oqT[:, j, 128 * t:128 * (t + 1)],
                                wo_sb[:, j, ck:ck + 512],
                                start=(j == 0), stop=(j == 7))
                        at5 = p5.tile([128, 512], F32, tag="at5")
                        nc.scalar.activation(out=at5, in_=pw,
                                             func=AF.Identity,
                                             scale=dqOo[:, t:t + 1])
                        ug = p5.tile([128, 512], F32, tag="ug")
                        nc.vector.tensor_tensor(out=ug, in0=at5,
                                                in1=B_g1[:, ck:ck + 512],
                                                op=AL.mult)
                        nc.vector.tensor_tensor(out=xn[:, ck:ck + 512],
                                                in0=ug,
                                                in1=xr2[:, ck:ck + 512],
                                                op=AL.add)
                    st = p5.tile([128, 2, 6], F32, tag="bst2")
                    xrr = xn.rearrange("p (s d) -> p s d", s=2)
                    for s2 in range(2):
                        nc.vector.bn_stats(out=st[:, s2, :], in_=xrr[:, s2, :])
                    mv = p5.tile([128, 2], F32, tag="bmv2")
                    nc.vector.bn_aggr(out=mv, in_=st)
                    rstdC = newton_rsqrt(p5, mv[:, 1:2], 1.0, 1e-6, [128, 1],
                                         "rC")
                    nmrC = p5.tile([128, 1], F32, tag="nmrC")
                    nc.vector.tensor_tensor(out=nmrC, in0=mv[:, 0:1],
                                            in1=rstdC, op=AL.mult)
                    nc.vector.tensor_scalar(out=nmrC, in0=nmrC, scalar1=-1.0,
                                            scalar2=None, op0=AL.mult)
                    u2 = p5.tile([128, D], F32, tag="u2")
                    nc.scalar.activation(out=u2, in_=xn, func=AF.Identity,
                                         scale=rstdC, bias=nmrC)
                    tt2 = p5.tile([128, D], F32, tag="tt2")
                    nc.vector.tensor_tensor(out=tt2, in0=u2, in1=B_sc2,
                                            op=AL.mult)
                    mod2 = p5.tile([128, D], F32, tag="mod2", bufs=2)
                    nc.vector.tensor_tensor(out=mod2, in0=tt2, in1=B_sh2,
                                            op=AL.add)
                    nc.vector.tensor_reduce(out=amC[:, t:t + 1], in_=mod2,
                                            axis=AX.X, op=AL.max,
                                            apply_absolute_value=True)
                    sqc = p5.tile([128, D], BF, tag="sqc")
                    nc.scalar.activation(out=sqc, in_=mod2, func=AF.Square,
                                         accum_out=ssC[:, t:t + 1])
                    quant_cols(p5, amC[:, t:t + 1], ssC[:, t:t + 1], D,
                               q127C[:, t:t + 1], dqCg[:, t:t + 1], iw["gate"],
                               [128, 1], "qc")
                    kq = round_bf16(p5, mod2, q127C[:, t:t + 1], "rc")
                    nc.sync.dma_start(out=x2qT[:, :, 128 * t:128 * (t + 1)],
                                      in_=kq, transpose=True)
            pO.__exit__(None, None, None)   # free oqT/wo
            pG.__exit__(None, None, None)   # free gs

            # =========== phase 6: MLP gate matmuls -> h2 (bf16) ==========
            pDW = tc.tile_pool(name="pDW", bufs=1)
            pdw = pDW.__enter__()
            dw_sb = pdw.tile([128, 32, D], BF, tag="dwsb", bufs=1)
            nc.gpsimd.dma_start(
                out=dw_sb,
                in_=dwT[:, :].rearrange("(a p) q -> p a q", p=128))
            with tc.tile_pool(name="p6", bufs=2) as p6:
                amDg = p6.tile([128, 8, 8], F32, tag="amDg", bufs=1)
                ssDg = p6.tile([128, 8, 8], F32, tag="ssDg", bufs=1)
                for g in range(8):
                    gw_g = p6.tile([128, 8, 1024], BF, tag="gwg")
                    nc.gpsimd.dma_start(
                        out=gw_g[:, :, 0:512],
                        in_=gwT[:, 512 * g:512 * (g + 1)].rearrange(
                            "(a p) q -> p a q", p=128))
                    nc.gpsimd.dma_start(
                        out=gw_g[:, :, 512:1024],
                        in_=gwT[:, MLP + 512 * g:MLP + 512 * (g + 1)].rearrange(
                            "(a p) q -> p a q", p=128))
                    for t in range(8):
                        pg = pmm()
                        py = pmm()
                        for j in range(8):
                            nc.tensor.matmul(
                                pg, x2qT[:, j, 128 * t:128 * (t + 1)],
                                gw_g[:, j, 0:512],
                                start=(j == 0), stop=(j == 7))
                        for j in range(8):
                            nc.tensor.matmul(
                                py, x2qT[:, j, 128 * t:128 * (t + 1)],
                                gw_g[:, j, 512:1024],
                                start=(j == 0), stop=(j == 7))
                        sil = p6.tile([128, 512], F32, tag="sil")
                        nc.scalar.activation(out=sil, in_=pg, func=AF.Silu,
                                             scale=dqCg[:, t:t + 1])
                        h2c = p6.tile([128, 512], BF, tag="h2c", bufs=3)
                        nc.vector.scalar_tensor_tensor(
                            out=h2c, in0=py, scalar=dqCg[:, t:t + 1],
                            in1=sil, op0=AL.mult, op1=AL.mult)
                        nc.sync.dma_start(
                            out=h2_d[128 * t:128 * (t + 1),
                                     512 * g:512 * (g + 1)],
                            in_=h2c)
                        nc.vector.tensor_reduce(out=amDg[:, t, g:g + 1],
                                                in_=h2c, axis=AX.X, op=AL.max,
                                                apply_absolute_value=True)
                        sqd = p6.tile([128, 512], BF, tag="sqd")
                        nc.scalar.activation(out=sqd, in_=h2c, func=AF.Square,
                                             accum_out=ssDg[:, t, g:g + 1])
                amD = p6.tile([128, 8], F32, tag="amD", bufs=1)
                ssD = p6.tile([128, 8], F32, tag="ssD", bufs=1)
                nc.vector.tensor_reduce(out=amD, in_=amDg, axis=AX.X,
                                        op=AL.max)
                nc.vector.tensor_reduce(out=ssD, in_=ssDg, axis=AX.X,
                                        op=AL.add)
                quant_cols(p6, amD, ssD, MLP, q127D, dqDo, iw["down"],
                           [128, 8], "qd")

            # ====== phase 7: round h2 + down matmuls + final residual ====
            with tc.tile_pool(name="p7", bufs=2) as p7:
                for t in range(8):
                    h2t = p7.tile([128, 32, 128], BF, tag="h2t")
                    for q in range(4):
                        h2r = p7.tile([128, D], BF, tag="h2r", bufs=3)
                        nc.scalar.dma_start(
                            out=h2r,
                            in_=h2_d[128 * t:128 * (t + 1),
                                     1024 * q:1024 * (q + 1)])
                        kqd = round_bf16(p7, h2r, q127D[:, t:t + 1], "rd",
                                         bufs=2)
                        nc.sync.dma_start(out=h2t[:, 8 * q:8 * (q + 1), :],
                                          in_=kqd, transpose=True)
                    outt = p7.tile([128, D], F32, tag="outt")
                    for ck in range(0, D, 512):
                        pdn = pmm()
                        for j2 in range(32):
                            nc.tensor.matmul(pdn, h2t[:, j2, :],
                                             dw_sb[:, j2, ck:ck + 512],
                                             start=(j2 == 0), stop=(j2 == 31))
                        at7 = p7.tile([128, 512], F32, tag="at7")
                        nc.scalar.activation(out=at7, in_=pdn,
                                             func=AF.Identity,
                                             scale=dqDo[:, t:t + 1])
                        uv = p7.tile([128, 512], F32, tag="uv")
                        nc.vector.tensor_tensor(out=uv, in0=at7,
                                                in1=B_g2[:, ck:ck + 512],
                                                op=AL.mult)
                        nc.vector.tensor_tensor(out=outt[:, ck:ck + 512],
                                                in0=uv,
                                                in1=xnew[:, t, ck:ck + 512],
                                                op=AL.add)
                    nc.scalar.dma_start(out=out_sl[128 * t:128 * (t + 1), :],
                                        in_=outt)
            pDW.__exit__(None, None, None)

    nc.finalize()
    return nc


@functools.lru_cache(maxsize=2)
def _build_cached(iw_items):
    return _build(dict(iw_items))


def kernel(x, c, adaln_w, adaln_b, wi, wf, wg, gnorm_w, wo, gate_w, down_w):
    x = np.ascontiguousarray(np.asarray(x, dtype=np.float32))
    c = np.ascontiguousarray(np.asarray(c, dtype=np.float32))
    adaln_w = np.asarray(adaln_w, dtype=np.float32)
    adaln_b = np.asarray(adaln_b, dtype=np.float32)
    gnorm_w = np.asarray(gnorm_w, dtype=np.float32)

    mi, iwi = _quant_w(np.asarray(wi, dtype=np.float32))
    mf, iwf = _quant_w(np.asarray(wf, dtype=np.float32))
    mg, iwg = _quant_w(np.asarray(wg, dtype=np.float32))
    mo, iwo = _quant_w(np.asarray(wo, dtype=np.float32))
    mgate, iwgate = _quant_w(np.asarray(gate_w, dtype=np.float32))
    mdown, iwdown = _quant_w(np.asarray(down_w, dtype=np.float32))

    iw = {"i": float(iwi), "f": float(iwf), "g": float(iwg), "o": float(iwo),
          "gate": float(iwgate), "down": float(iwdown)}
    nc = _build_cached(tuple(sorted(iw.items())))

    wiT_h = np.ascontiguousarray(mi.T)
    wfT_h = np.ascontiguousarray(mf.T)
    wgT_h = np.ascontiguousarray(mg.T)
    woT_h = np.ascontiguousarray(mo.T)
    gwT_h = np.ascontiguousarray(mgate.T)
    dwT_h = np.ascontiguousarray(mdown.T)
    adwT = np.ascontiguousarray(adaln_w.T)          # [D, 6D]
    gnr_h = np.ascontiguousarray(np.tile(gnorm_w, NH)[None, :])
    c_cols_h = np.ascontiguousarray(
        c.T.reshape(8, 128, B).transpose(1, 0, 2))   # [128, 8, B]

    in_maps = []
    for core in range(N_CORES):
        b, half = core // 2, core % 2
        mask = np.zeros((N_CORES, 1), np.float32)
        if half == 1:
            mask[core - 1, 0] = 1.0
        bm = np.zeros((B, 1), np.float32)
        bm[b, 0] = 1.0
        in_maps.append({
            "x_sl": np.ascontiguousarray(x[b, half * TOK:(half + 1) * TOK, :]),
            "c_cols": c_cols_h,
            "adw_sl": np.ascontiguousarray(adwT[:, 768 * core:768 * (core + 1)]),
            "adb_sl": np.ascontiguousarray(
                adaln_b[None, 768 * core:768 * (core + 1)]),
            "mask8": mask,
            "bsel": np.ones((1, B), np.float32),
            "bmask": bm,
            "gnr": gnr_h,
            "wiT": wiT_h, "wfT": wfT_h, "wgT": wgT_h, "woT": woT_h,
            "gwT": gwT_h, "dwT": dwT_h,
        })

    res = run_bass_kernel_spmd(nc, in_maps, core_ids=list(range(N_CORES)))
    out = np.zeros((B, T, D), np.float32)
    for core in range(N_CORES):
        b, half = core // 2, core % 2
        out[b, half * TOK:(half + 1) * TOK, :] = res.results[core]["out_sl"]
    return out


# revision 36
# speedup vs baseline: 1.0120x; 1.0120x over previous
"""Trainium2 Bass kernel for nn_DiTBlock (HGRN-attention DiT block).

Sharding: 8 cores = 4 batches x 2 half-sequences (1024 tokens each).
All matmuls are exact-integer bf16 matmuls (activations quantized to
int8-range integers in bf16; ternary weights quantized on host).

v6 structure:
  - per-token-tile pipelining (LN/quant/round per 128-token tile)
  - activation transposes via DMA-XBAR (bf16); only the f32 h-transpose
    uses the PE
  - AllGathers overlapped (adaln AG under the LN-stats sweep, the scan
    carry AG under interleaved wg matmuls)
  - exact fp32 C_MAGIC rounding; subtract step on ScalarE
  - scan outputs (ha/cam) spilled to DRAM per chunk, reloaded for the
    carry fix; h2 spilled pre-round as bf16
  - weight loads on the scalar/gpsimd HWDGE queues so they never queue
    behind compute-dependent DMAs
"""
import functools
import numpy as np
import ml_dtypes

import concourse.bass as bass
import concourse.bacc as bacc_mod
import concourse.mybir as mybir
import concourse.tile as tile
from concourse.masks import make_identity
from concourse.bass_utils import run_bass_kernel_spmd

BF16 = ml_dtypes.bfloat16
F32 = mybir.dt.float32
BF = mybir.dt.bfloat16
U32 = mybir.dt.uint32
AL = mybir.AluOpType
AF = mybir.ActivationFunctionType
AX = mybir.AxisListType

B, T, D = 4, 2048, 1024
TOK = 1024          # tokens per core
NH, HD = 16, 64
MLP = 4096
N_CORES = 8
CMAGIC = float(1.5 * 2 ** 23)
MAGIC_U32 = 0x5F3759DF


def _quant_w(w):
    invws = float(np.clip(np.abs(w).mean(dtype=np.float64), 1e-5, None))
    m = np.clip(np.round(w.astype(np.float64) / invws), -1, 1).astype(np.float32)
    return np.ascontiguousarray(m.astype(BF16)), np.float32(invws)


def _build(iw):
    nc = bacc_mod.Bacc("TRN2", target_bir_lowering=False)

    x_sl = nc.declare_dram_parameter("x_sl", [TOK, D], F32, isOutput=False)
    c_cols = nc.declare_dram_parameter("c_cols", [128, 8, B], F32, isOutput=False)
    adw_sl = nc.declare_dram_parameter("adw_sl", [D, 768], F32, isOutput=False)
    adb_sl = nc.declare_dram_parameter("adb_sl", [1, 768], F32, isOutput=False)
    mask8 = nc.declare_dram_parameter("mask8", [N_CORES, 1], F32, isOutput=False)
    bsel = nc.declare_dram_parameter("bsel", [1, B], F32, isOutput=False)
    bmask = nc.declare_dram_parameter("bmask", [B, 1], F32, isOutput=False)
    gnr = nc.declare_dram_parameter("gnr", [1, D], F32, isOutput=False)
    wiT = nc.declare_dram_parameter("wiT", [D, D], BF, isOutput=False)
    wfT = nc.declare_dram_parameter("wfT", [D, D], BF, isOutput=False)
    wgT = nc.declare_dram_parameter("wgT", [D, D], BF, isOutput=False)
    woT = nc.declare_dram_parameter("woT", [D, D], BF, isOutput=False)
    gwT = nc.declare_dram_parameter("gwT", [D, 2 * MLP], BF, isOutput=False)
    dwT = nc.declare_dram_parameter("dwT", [MLP, D], BF, isOutput=False)
    out_sl = nc.declare_dram_parameter("out_sl", [TOK, D], F32, isOutput=True)

    cc1_in = nc.dram_tensor("cc1_in", [B, 768], F32)
    cc1_out = nc.dram_tensor("cc1_out", [N_CORES * B, 768], F32,
                             addr_space="Shared")
    cc2_in = nc.dram_tensor("cc2_in", [D], F32)
    cc2_out = nc.dram_tensor("cc2_out", [N_CORES, D], F32, addr_space="Shared")

    RG = [list(range(N_CORES))]

    with tile.TileContext(nc) as tc:
        import contextlib
        es = contextlib.ExitStack()
        with es:
            cst = es.enter_context(tc.tile_pool(name="cst", bufs=1))
            ps = es.enter_context(tc.tile_pool(name="ps", bufs=4, space="PSUM"))
            psm = es.enter_context(tc.tile_pool(name="psm", bufs=1, space="PSUM"))
            pst = es.enter_context(tc.tile_pool(name="pst", bufs=2, space="PSUM"))
            dr = es.enter_context(tc.tile_pool(name="dr", bufs=1, space="DRAM"))

            def pmm():
                return ps.tile([128, 512], F32, tag="mm", name="mm")

            def newton_rsqrt(sb, x_ap, scale, bias, shape, tag, iters=3):
                """rsqrt(x*scale + bias) via bit-magic + Newton iters."""
                t = sb.tile(shape, F32, tag=tag + "_t", name=tag + "_t")
                nc.vector.tensor_scalar(out=t, in0=x_ap, scalar1=float(scale),
                                        scalar2=float(bias), op0=AL.mult,
                                        op1=AL.add)
                y = sb.tile(shape, F32, tag=tag + "_y", name=tag + "_y")
                sh = sb.tile(shape, F32, tag=tag + "_s", name=tag + "_s")
                nc.vector.tensor_scalar(out=sh[:].bitcast(U32),
                                        in0=t[:].bitcast(U32), scalar1=1,
                                        scalar2=None,
                                        op0=AL.logical_shift_right)
                mg = sb.tile(shape, F32, tag=tag + "_m", name=tag + "_m")
                nc.vector.memset(mg[:].bitcast(U32), MAGIC_U32)
                nc.vector.tensor_tensor(out=y[:].bitcast(U32),
                                        in0=mg[:].bitcast(U32),
                                        in1=sh[:].bitcast(U32), op=AL.subtract)
                e = sb.tile(shape, F32, tag=tag + "_e", name=tag + "_e")
                for _ in range(iters):
                    nc.vector.tensor_tensor(out=e, in0=y, in1=y, op=AL.mult)
                    nc.vector.tensor_tensor(out=e, in0=e, in1=t, op=AL.mult)
                    nc.vector.tensor_scalar(out=e, in0=e, scalar1=-0.5,
                                            scalar2=1.5, op0=AL.mult,
                                            op1=AL.add)
                    nc.vector.tensor_tensor(out=y, in0=y, in1=e, op=AL.mult)
                return y

            def quant_cols(sb, am_col, ss_col, dk, q_col, dq_col, dq_scale,
                           shape, tag):
                """q = 127/max(am,1e-5); dq = am*rsqrt(ss/dk+1e-8)*scale/127"""
                amc = sb.tile(shape, F32, tag=tag + "amc", name=tag + "amc")
                nc.vector.tensor_scalar(out=amc, in0=am_col, scalar1=1e-5,
                                        scalar2=None, op0=AL.max)
                rec = sb.tile(shape, F32, tag=tag + "rec", name=tag + "rec")
                nc.vector.reciprocal(out=rec, in_=amc)
                nc.vector.tensor_scalar(out=q_col, in0=rec, scalar1=127.0,
                                        scalar2=None, op0=AL.mult)
                rs = newton_rsqrt(sb, ss_col, 1.0 / dk, 1e-8, shape,
                                  tag + "rs", iters=2)
                nc.vector.tensor_tensor(out=dq_col, in0=amc, in1=rs,
                                        op=AL.mult)
                nc.vector.tensor_scalar(out=dq_col, in0=dq_col,
                                        scalar1=float(dq_scale) / 127.0,
                                        scalar2=None, op0=AL.mult)

            # ---------------- consts (whole-kernel lifetime) -------------
            identf = cst.tile([128, 128], F32)
            make_identity(nc, identf)
            ones_row = cst.tile([1, 128], F32)
            nc.vector.memset(ones_row, 1.0)
            negmagic = cst.tile([128, 1], F32)
            nc.vector.memset(negmagic, -CMAGIC)
            mask_sb = cst.tile([N_CORES, 1], F32)
            nc.sync.dma_start(out=mask_sb, in_=mask8[:, :])
            bsel_sb = cst.tile([1, B], F32)
            nc.sync.dma_start(out=bsel_sb, in_=bsel[:, :])
            bmask_sb = cst.tile([B, 1], F32)
            nc.sync.dma_start(out=bmask_sb, in_=bmask[:, :])

            def round_bf16(sb, src, q_col, tagp, bufs=3):
                """round(src*q) -> integer-valued bf16 tile via fp32 magic."""
                t2 = sb.tile([128, src.free_size()], F32, tag=tagp + "t2",
                             name=tagp + "t2")
                nc.vector.tensor_scalar(out=t2, in0=src, scalar1=q_col,
                                        scalar2=CMAGIC, op0=AL.mult,
                                        op1=AL.add)
                kq = sb.tile([128, src.free_size()], BF, tag=tagp + "kq",
                             name=tagp + "kq", bufs=bufs)
                nc.scalar.activation(out=kq, in_=t2, func=AF.Identity,
                                     bias=negmagic)
                return kq

            q127A = cst.tile([128, 8], F32); dqA = cst.tile([128, 8], F32)
            dqAg = cst.tile([128, 8], F32)
            q127O = cst.tile([128, 8], F32); dqOo = cst.tile([128, 8], F32)
            q127C = cst.tile([128, 8], F32); dqCg = cst.tile([128, 8], F32)
            q127D = cst.tile([128, 8], F32); dqDo = cst.tile([128, 8], F32)
            B_gn = cst.tile([128, D], F32)
            B_g1 = cst.tile([128, D], F32)
            B_sh2 = cst.tile([128, D], F32)
            B_sc2 = cst.tile([128, D], F32)
            B_g2 = cst.tile([128, D], F32)

            params_d = dr.tile([6 * D], F32, tag="params")
            ha_d = dr.tile([TOK, D], F32, tag="had")
            cam_d = dr.tile([TOK, D], BF, tag="camd")
            h2_d = dr.tile([TOK, MLP], BF, tag="h2d")

            # whole-kernel SBUF: xqT (-> x2qT) and xnew
            big = es.enter_context(tc.tile_pool(name="big", bufs=1))
            xqT = big.tile([128, 8, TOK], BF, tag="xq", bufs=1)

            def bcast_row(row_ap, dst, plus1=False):
                for ch in range(0, D, 512):
                    pb = pst.tile([128, 512], F32, tag="aux", name="aux")
                    nc.tensor.matmul(pb, ones_row, row_ap[:, ch:ch + 512],
                                     start=True, stop=True)
                    if plus1:
                        nc.scalar.activation(out=dst[:, ch:ch + 512], in_=pb,
                                             func=AF.Identity, bias=1.0)
                    else:
                        nc.scalar.copy(out=dst[:, ch:ch + 512], in_=pb)

            # =========== phase 0: adaln (+AG) and LN stats ===========
            with tc.tile_pool(name="p0", bufs=2) as p0:
                B_sh1 = p0.tile([128, D], F32, tag="Bsh1", bufs=1)
                B_sc1 = p0.tile([128, D], F32, tag="Bsc1", bufs=1)
                gnr_sb = p0.tile([1, D], F32, tag="gnr", bufs=1)
                nc.sync.dma_start(out=gnr_sb, in_=gnr[:, :])

                # adaln: this core computes 768 of the 6144 outputs
                c_sb = p0.tile([128, 8, B], F32, tag="csb", bufs=1)
                nc.sync.dma_start(out=c_sb, in_=c_cols[:, :, :])
                cs_sb = p0.tile([128, 8, B], F32, tag="cssb", bufs=1)
                nc.scalar.activation(out=cs_sb, in_=c_sb, func=AF.Silu)
                adb_sb = p0.tile([1, 768], F32, tag="adb", bufs=1)
                nc.sync.dma_start(out=adb_sb, in_=adb_sl[:, :])
                psA = psm.tile([B, 512], F32, tag="sm")
                psB = psm.tile([B, 256], F32, tag="sm2")
                for j in range(8):
                    adw_j = p0.tile([128, 768], F32, tag="adw")
                    nc.sync.dma_start(out=adw_j,
                                      in_=adw_sl[128 * j:128 * (j + 1), :])
                    nc.tensor.matmul(psA, cs_sb[:, j, :], adw_j[:, 0:512],
                                     start=(j == 0), stop=False)
                    nc.tensor.matmul(psB, cs_sb[:, j, :], adw_j[:, 512:768],
                                     start=(j == 0), stop=False)
                nc.tensor.matmul(psA, bsel_sb, adb_sb[:, 0:512],
                                 start=False, stop=True)
                nc.tensor.matmul(psB, bsel_sb, adb_sb[:, 512:768],
                                 start=False, stop=True)
                ad_sb = p0.tile([B, 768], F32, tag="adsb", bufs=1)
                nc.scalar.copy(out=ad_sb[:, 0:512], in_=psA)
                nc.scalar.copy(out=ad_sb[:, 512:768], in_=psB)
                nc.sync.dma_start(out=cc1_in[:, :], in_=ad_sb)
                nc.gpsimd.collective_compute(
                    "AllGather", AL.bypass, ins=[cc1_in[:]],
                    outs=[cc1_out[:]], replica_groups=RG)

                # LN stats pass (params-independent -> overlaps the AG)
                amA = p0.tile([128, 8], F32, tag="amA", bufs=1)
                ssA = p0.tile([128, 8], F32, tag="ssA", bufs=1)
                rstdA = p0.tile([128, 8], F32, tag="rstdA", bufs=1)
                nmrA = p0.tile([128, 8], F32, tag="nmrA", bufs=1)
                for i in range(8):
                    xt = p0.tile([128, D], F32, tag="xt", bufs=2)
                    nc.sync.dma_start(out=xt,
                                      in_=x_sl[128 * i:128 * (i + 1), :])
                    st = p0.tile([128, 2, 6], F32, tag="bst")
                    xr = xt.rearrange("p (s d) -> p s d", s=2)
                    for s2 in range(2):
                        nc.vector.bn_stats(out=st[:, s2, :], in_=xr[:, s2, :])
                    mv = p0.tile([128, 2], F32, tag="bmv")
                    nc.vector.bn_aggr(out=mv, in_=st)
                    rstdLN = newton_rsqrt(p0, mv[:, 1:2], 1.0, 1e-6,
                                          [128, 1], "rLN")
                    nc.vector.tensor_copy(out=rstdA[:, i:i + 1], in_=rstdLN)
                    nc.vector.tensor_tensor(out=nmrA[:, i:i + 1],
                                            in0=mv[:, 0:1], in1=rstdLN,
                                            op=AL.mult)
                nc.vector.tensor_scalar(out=nmrA, in0=nmrA, scalar1=-1.0,
                                        scalar2=None, op0=AL.mult)

                # select this batch's row per 768-block, stage through DRAM
                for r in range(8):
                    ag_r = p0.tile([B, 768], F32, tag="ag1")
                    nc.sync.dma_start(out=ag_r,
                                      in_=cc1_out[4 * r:4 * (r + 1), :])
                    pp1 = psm.tile([1, 512], F32, tag="sm")
                    pp2 = psm.tile([1, 256], F32, tag="sm2")
                    nc.tensor.matmul(pp1, bmask_sb, ag_r[:, 0:512],
                                     start=True, stop=True)
                    nc.tensor.matmul(pp2, bmask_sb, ag_r[:, 512:768],
                                     start=True, stop=True)
                    rb7 = p0.tile([1, 768], F32, tag="rb7", bufs=1)
                    nc.scalar.copy(out=rb7[:, 0:512], in_=pp1)
                    nc.scalar.copy(out=rb7[:, 512:768], in_=pp2)
                    nc.sync.dma_start(
                        out=params_d[768 * r:768 * (r + 1)].rearrange(
                            "(one c) -> one c", one=1),
                        in_=rb7)
                B_list = [(B_sh1, False), (B_sc1, True), (B_g1, False),
                          (B_sh2, False), (B_sc2, True), (B_g2, False)]
                for k, (dst, plus1) in enumerate(B_list):
                    rbk = p0.tile([1, D], F32, tag="rbk", bufs=1)
                    nc.sync.dma_start(
                        out=rbk,
                        in_=params_d[D * k:D * (k + 1)].rearrange(
                            "(one c) -> one c", one=1))
                    bcast_row(rbk, dst, plus1=plus1)
                bcast_row(gnr_sb, B_gn)

                # ===== phase 1: modulate + quant + round (per tile) =====
                for i in range(8):
                    xt = p0.tile([128, D], F32, tag="xt2", bufs=2)
                    nc.sync.dma_start(out=xt,
                                      in_=x_sl[128 * i:128 * (i + 1), :])
                    u = p0.tile([128, D], F32, tag="u")
                    nc.scalar.activation(out=u, in_=xt, func=AF.Identity,
                                         scale=rstdA[:, i:i + 1],
                                         bias=nmrA[:, i:i + 1])
                    ttm = p0.tile([128, D], F32, tag="ttm")
                    nc.vector.tensor_tensor(out=ttm, in0=u, in1=B_sc1,
                                            op=AL.mult)
                    moda = p0.tile([128, D], F32, tag="moda", bufs=2)
                    nc.vector.tensor_tensor(out=moda, in0=ttm, in1=B_sh1,
                                            op=AL.add)
                    nc.vector.tensor_reduce(out=amA[:, i:i + 1], in_=moda,
                                            axis=AX.X, op=AL.max,
                                            apply_absolute_value=True)
                    sqs = p0.tile([128, D], BF, tag="sqs")
                    nc.scalar.activation(out=sqs, in_=moda, func=AF.Square,
                                         accum_out=ssA[:, i:i + 1])
                    quant_cols(p0, amA[:, i:i + 1], ssA[:, i:i + 1], D,
                               q127A[:, i:i + 1], dqA[:, i:i + 1], 1.0,
                               [128, 1], "qa")
                    nc.vector.tensor_scalar(out=dqAg[:, i:i + 1],
                                            in0=dqA[:, i:i + 1],
                                            scalar1=float(iw["g"]),
                                            scalar2=None, op0=AL.mult)
                    kq = round_bf16(p0, moda, q127A[:, i:i + 1], "ra")
                    nc.sync.dma_start(out=xqT[:, :, 128 * i:128 * (i + 1)],
                                      in_=kq, transpose=True)

            # ====== phase 2: i/f + g matmuls + scan (chunked) ======
            pG = tc.tile_pool(name="pG", bufs=1)
            pgs = pG.__enter__()
            gs = pgs.tile([128, 8, D], F32, tag="gs", bufs=1)
            with tc.tile_pool(name="p2", bufs=2) as p2:
                # Sb_i / Sb_f: dq row broadcast over partitions, scaled by iw
                Sb_i = p2.tile([128, TOK], F32, tag="Sbi", bufs=1)
                Sb_f = p2.tile([128, TOK], F32, tag="Sbf", bufs=1)
                dqrow_sb = p2.tile([1, D], F32, tag="dqrow", bufs=1)
                for i8 in range(8):
                    ptr = psm.tile([1, 128], F32, tag="sm")
                    nc.tensor.transpose(ptr, dqA[:, i8:i8 + 1], identf)
                    nc.scalar.copy(out=dqrow_sb[:, 128 * i8:128 * (i8 + 1)],
                                   in_=ptr)
                oi = p2.tile([1, 128], F32, tag="oi", bufs=1)
                nc.vector.memset(oi, float(iw["i"]))
                of = p2.tile([1, 128], F32, tag="of", bufs=1)
                nc.vector.memset(of, float(iw["f"]))
                for ch in range(0, TOK, 512):
                    pb = pst.tile([128, 512], F32, tag="aux", name="aux")
                    nc.tensor.matmul(pb, oi, dqrow_sb[:, ch:ch + 512],
                                     start=True, stop=True)
                    nc.scalar.copy(out=Sb_i[:, ch:ch + 512], in_=pb)
                    pb2 = pst.tile([128, 512], F32, tag="aux", name="aux")
                    nc.tensor.matmul(pb2, of, dqrow_sb[:, ch:ch + 512],
                                     start=True, stop=True)
                    nc.scalar.copy(out=Sb_f[:, ch:ch + 512], in_=pb2)

                wf_sb = p2.tile([128, 8, D], BF, tag="wfsb", bufs=1)
                nc.scalar.dma_start(
                    out=wf_sb,
                    in_=wfT[:, :].rearrange("(a p) q -> p a q", p=128))
                wi_sb = p2.tile([128, 8, D], BF, tag="wisb", bufs=1)
                nc.scalar.dma_start(
                    out=wi_sb,
                    in_=wiT[:, :].rearrange("(a p) q -> p a q", p=128))
                wg_sb = p2.tile([128, 8, D], BF, tag="wgsb", bufs=1)
                nc.scalar.dma_start(
                    out=wg_sb,
                    in_=wgT[:, :].rearrange("(a p) q -> p a q", p=128))

                def g_mms(trange):
                    for t in trange:
                        for ck2 in range(0, D, 512):
                            pg = pmm()
                            for j in range(8):
                                nc.tensor.matmul(
                                    pg, xqT[:, j, 128 * t:128 * (t + 1)],
                                    wg_sb[:, j, ck2:ck2 + 512],
                                    start=(j == 0), stop=(j == 7))
                            scr = p2.tile([128, 512], F32, tag="gscr",
                                          bufs=2)
                            nc.scalar.activation(out=scr, in_=pg,
                                                 func=AF.Silu,
                                                 scale=dqAg[:, t:t + 1])
                            nc.vector.tensor_tensor(
                                out=gs[:, t, ck2:ck2 + 512], in0=scr,
                                in1=B_gn[:, ck2:ck2 + 512], op=AL.mult)

                ha_last = p2.tile([128, 8], F32, tag="halast", bufs=1)
                cam_last = p2.tile([128, 8], F32, tag="camlast", bufs=1)

                def scan_chain(ft, it, m, ck, cki):
                    sigf = p2.tile([128, 512], F32, tag="sigf", bufs=2)
                    nc.scalar.activation(out=sigf, in_=ft, func=AF.Sigmoid)
                    omf = p2.tile([128, 512], F32, tag="omf", bufs=2)
                    nc.scalar.activation(out=omf, in_=ft, func=AF.Sigmoid,
                                         scale=-1.0)
                    sili = p2.tile([128, 512], F32, tag="sili", bufs=2)
                    nc.scalar.activation(out=sili, in_=it, func=AF.Silu)
                    ifin = p2.tile([128, 512], F32, tag="ifin", bufs=2)
                    nc.vector.tensor_tensor(out=ifin, in0=sili, in1=omf,
                                            op=AL.mult)
                    ha_c = p2.tile([128, 512], F32, tag="hac", bufs=3)
                    init_h = 0.0 if cki == 0 else ha_last[:, m:m + 1]
                    nc.vector.tensor_tensor_scan(
                        ha_c, sigf, ifin, init_h, op0=AL.mult, op1=AL.add)
                    cam_c = p2.tile([128, 512], BF, tag="camc", bufs=3)
                    init_c = 1.0 if cki == 0 else cam_last[:, m:m + 1]
                    nc.vector.tensor_tensor_scan(
                        cam_c, sigf, sigf, init_c, op0=AL.mult, op1=AL.bypass)
                    nc.sync.dma_start(
                        out=ha_d[128 * m:128 * (m + 1), ck:ck + 512],
                        in_=ha_c)
                    nc.sync.dma_start(
                        out=cam_d[128 * m:128 * (m + 1), ck:ck + 512],
                        in_=cam_c)
                    if cki == 0:
                        nc.vector.tensor_copy(out=ha_last[:, m:m + 1],
                                              in_=ha_c[:, 511:512])
                        nc.vector.tensor_copy(out=cam_last[:, m:m + 1],
                                              in_=cam_c[:, 511:512])
                    else:
                        nc.sync.dma_start(
                            out=cc2_in[128 * m:128 * (m + 1)].rearrange(
                                "(p one) -> p one", one=1),
                            in_=ha_c[:, 511:512])

                for cki, ck in enumerate(range(0, TOK, 512)):
                    for m in range(8):
                        pf = pmm()
                        pi = pmm()
                        for j in range(8):
                            nc.tensor.matmul(
                                pf, wf_sb[:, j, 128 * m:128 * (m + 1)],
                                xqT[:, j, ck:ck + 512],
                                start=(j == 0), stop=(j == 7))
                        for j in range(8):
                            nc.tensor.matmul(
                                pi, wi_sb[:, j, 128 * m:128 * (m + 1)],
                                xqT[:, j, ck:ck + 512],
                                start=(j == 0), stop=(j == 7))
                        ft = p2.tile([128, 512], F32, tag="ftm", bufs=2)
                        nc.vector.tensor_tensor(out=ft, in0=pf,
                                                in1=Sb_f[:, ck:ck + 512],
                                                op=AL.mult)
                        it = p2.tile([128, 512], F32, tag="itm", bufs=2)
                        nc.vector.tensor_tensor(out=it, in0=pi,
                                                in1=Sb_i[:, ck:ck + 512],
                                                op=AL.mult)
                        scan_chain(ft, it, m, ck, cki)
                    g_mms(range(0, 4) if cki == 0 else range(4, 8))
                nc.gpsimd.collective_compute(
                    "AllGather", AL.bypass, ins=[cc2_in[:]], outs=[cc2_out[:]],
                    replica_groups=RG)

            # =========== phase 3: carry fix + hT (PE transpose) ==========
            pO = tc.tile_pool(name="pO", bufs=1)
            pos_ = pO.__enter__()
            oqT = pos_.tile([128, 8, D], BF, tag="oqT", bufs=1)
            wo_sb = pos_.tile([128, 8, D], BF, tag="wosb", bufs=1)
            nc.scalar.dma_start(
                out=wo_sb,
                in_=woT[:, :].rearrange("(a p) q -> p a q", p=128))
            pH3 = tc.tile_pool(name="pH3", bufs=1)
            ph3_ = pH3.__enter__()
            hT = ph3_.tile([128, 8, D], F32, tag="hT", bufs=1)
            with tc.tile_pool(name="p3", bufs=2) as p3:
                ag2 = p3.tile([N_CORES, D], F32, tag="ag2", bufs=1)
                nc.sync.dma_start(out=ag2, in_=cc2_out[:, :])
                for m in range(8):
                    pc = psm.tile([128, 1], F32, tag="sm")
                    nc.tensor.matmul(pc, ag2[:, 128 * m:128 * (m + 1)],
                                     mask_sb, start=True, stop=True)
                    carry = p3.tile([128, 1], F32, tag="carry")
                    nc.scalar.copy(out=carry, in_=pc)
                    har = p3.tile([128, TOK], F32, tag="har", bufs=2)
                    nc.sync.dma_start(out=har,
                                      in_=ha_d[128 * m:128 * (m + 1), :])
                    camr = p3.tile([128, TOK], BF, tag="camr", bufs=2)
                    nc.sync.dma_start(out=camr,
                                      in_=cam_d[128 * m:128 * (m + 1), :])
                    hfix = p3.tile([128, TOK], F32, tag="hfix", bufs=2)
                    nc.vector.scalar_tensor_tensor(out=hfix, in0=camr,
                                                   scalar=carry, in1=har,
                                                   op0=AL.mult, op1=AL.add)
                    for g4 in range(0, 8, 4):
                        tp = pst.tile([128, 512], F32, tag="aux")
                        for jj in range(4):
                            t_i = g4 + jj
                            nc.tensor.transpose(
                                tp[:, 128 * jj:128 * (jj + 1)],
                                hfix[:, 128 * t_i:128 * (t_i + 1)], identf)
                        for jj in range(4):
                            t_i = g4 + jj
                            if jj % 2 == 0:
                                nc.scalar.copy(
                                    out=hT[:, t_i, 128 * m:128 * (m + 1)],
                                    in_=tp[:, 128 * jj:128 * (jj + 1)])
                            else:
                                nc.vector.tensor_copy(
                                    out=hT[:, t_i, 128 * m:128 * (m + 1)],
                                    in_=tp[:, 128 * jj:128 * (jj + 1)])

            # =========== phase 4: gnorm-swish-gate + o quant ===========
            with tc.tile_pool(name="p4", bufs=2) as p4:
                amO = p4.tile([128, 8], F32, tag="amO", bufs=1)
                ssO = p4.tile([128, 8], F32, tag="ssO", bufs=1)
                for t in range(8):
                    sq = p4.tile([128, D], F32, tag="sq")
                    nc.scalar.activation(out=sq, in_=hT[:, t, :],
                                         func=AF.Square)
                    msh = p4.tile([128, 16], F32, tag="msh")
                    nc.vector.tensor_reduce(
                        out=msh,
                        in_=sq.rearrange("p (h d) -> p h d", h=NH),
                        axis=AX.X, op=AL.add)
                    rstdH = newton_rsqrt(p4, msh, 1.0 / HD, 1e-5, [128, 16],
                                         "rH")
                    hn = p4.tile([128, D], F32, tag="hn")
                    rb = bass.AP(tensor=rstdH.tensor, offset=rstdH.offset,
                                 ap=[rstdH.ap[0], [1, NH], [0, HD]])
                    nc.vector.tensor_tensor(
                        out=hn.rearrange("p (h d) -> p h d", h=NH),
                        in0=hT[:, t, :].rearrange("p (h d) -> p h d", h=NH),
                        in1=rb, op=AL.mult)
                    oa = p4.tile([128, D], F32, tag="oa", bufs=2)
                    nc.vector.tensor_tensor(out=oa, in0=hn, in1=gs[:, t, :],
                                            op=AL.mult)
                    nc.vector.tensor_reduce(out=amO[:, t:t + 1], in_=oa,
                                            axis=AX.X, op=AL.max,
                                            apply_absolute_value=True)
                    sqo = p4.tile([128, D], BF, tag="sqo", bufs=1)
                    nc.scalar.activation(out=sqo, in_=oa, func=AF.Square,
                                         accum_out=ssO[:, t:t + 1])
                    quant_cols(p4, amO[:, t:t + 1], ssO[:, t:t + 1], D,
                               q127O[:, t:t + 1], dqOo[:, t:t + 1], iw["o"],
                               [128, 1], "qo")
                    kq = round_bf16(p4, oa, q127O[:, t:t + 1], "ro")
                    nc.sync.dma_start(out=oqT[:, :, 128 * t:128 * (t + 1)],
                                      in_=kq, transpose=True)
            pH3.__exit__(None, None, None)  # free hT

            # ====== phase 5: wo matmul + residual + LN2 + quant ======
            xnew = big.tile([128, 8, D], F32, tag="xnew", bufs=1)
            x2qT = big.tile([128, 8, TOK], BF, tag="xq", bufs=1)
            with tc.tile_pool(name="p5", bufs=2) as p5:
                amC = p5.tile([128, 8], F32, tag="amC", bufs=1)
                ssC = p5.tile([128, 8], F32, tag="ssC", bufs=1)
                for t in range(8):
                    xr2 = p5.tile([128, D], F32, tag="xr2", bufs=2)
                    nc.sync.dma_start(out=xr2,
                                      in_=x_sl[128 * t:128 * (t + 1), :])
                    xn = xnew[:, t, :]
                    for ck in range(0, D, 512):
                        pw = pmm()
                        for j in range(8):
                            nc.tensor.matmul(
                                pw, oqT[:, j, 128 * t:128 * (t + 1)],
                                wo_sb[:, j, ck:ck + 512],
                                start=(j == 0), stop=(j == 7))
                        ug = p5.tile([128, 512], F32, tag="ug")
                        nc.vector.scalar_tensor_tensor(
                            out=ug, in0=pw, scalar=dqOo[:, t:t + 1],
                            in1=B_g1[:, ck:ck + 512],
                            op0=AL.mult, op1=AL.mult)
                        nc.vector.tensor_tensor(out=xn[:, ck:ck + 512],
                                                in0=ug,
                                                in1=xr2[:, ck:ck + 512],
                                                op=AL.add)
                    st = p5.tile([128, 2, 6], F32, tag="bst2")
                    xrr = xn.rearrange("p (s d) -> p s d", s=2)
                    for s2 in range(2):
                        nc.vector.bn_stats(out=st[:, s2, :], in_=xrr[:, s2, :])
                    mv = p5.tile([128, 2], F32, tag="bmv2")
                    nc.vector.bn_aggr(out=mv, in_=st)
                    rstdC = newton_rsqrt(p5, mv[:, 1:2], 1.0, 1e-6, [128, 1],
                                         "rC")
                    nmrC = p5.tile([128, 1], F32, tag="nmrC")
                    nc.vector.tensor_tensor(out=nmrC, in0=mv[:, 0:1],
                                            in1=rstdC, op=AL.mult)
                    nc.vector.tensor_scalar(out=nmrC, in0=nmrC, scalar1=-1.0,
                                            scalar2=None, op0=AL.mult)
                    u2 = p5.tile([128, D], F32, tag="u2")
                    nc.scalar.activation(out=u2, in_=xn, func=AF.Identity,
                                         scale=rstdC, bias=nmrC)
                    tt2 = p5.tile([128, D], F32, tag="tt2")
                    nc.vector.tensor_tensor(out=tt2, in0=u2, in1=B_sc2,
                                            op=AL.mult)
                    mod2 = p5.tile([128, D], F32, tag="mod2", bufs=2)
                    nc.vector.tensor_tensor(out=mod2, in0=tt2, in1=B_sh2,
                                            op=AL.add)
                    nc.vector.tensor_reduce(out=amC[:, t:t + 1], in_=mod2,
                                            axis=AX.X, op=AL.max,
                                            apply_absolute_value=True)
                    sqc = p5.tile([128, D], BF, tag="sqc")
                    nc.scalar.activation(out=sqc, in_=mod2, func=AF.Square,
                                         accum_out=ssC[:, t:t + 1])
                    quant_cols(p5, amC[:, t:t + 1], ssC[:, t:t + 1], D,
                               q127C[:, t:t + 1], dqCg[:, t:t + 1], iw["gate"],
                               [128, 1], "qc")
                    kq = round_bf16(p5, mod2, q127C[:, t:t + 1], "rc")
                    nc.sync.dma_start(out=x2qT[:, :, 128 * t:128 * (t + 1)],
                                      in_=kq, transpose=True)
            pO.__exit__(None, None, None)   # free oqT/wo
            pG.__exit__(None, None, None)   # free gs

            # =========== phase 6: MLP gate matmuls -> h2 (bf16) ==========
            pDW = tc.tile_pool(name="pDW", bufs=1)
            pdw = pDW.__enter__()
            dw_sb = pdw.tile([128, 32, D], BF, tag="dwsb", bufs=1)
            nc.gpsimd.dma_start(
                out=dw_sb,
                in_=dwT[:, :].rearrange("(a p) q -> p a q", p=128))
            with tc.tile_pool(name="p6", bufs=2) as p6:
                amDg = p6.tile([128, 8, 8], F32, tag="amDg", bufs=1)
                ssDg = p6.tile([128, 8, 8], F32, tag="ssDg", bufs=1)
                for g in range(8):
                    gw_g = p6.tile([128, 8, 1024], BF, tag="gwg")
                    nc.gpsimd.dma_start(
                        out=gw_g[:, :, 0:512],
                        in_=gwT[:, 512 * g:512 * (g + 1)].rearrange(
                            "(a p) q -> p a q", p=128))
                    nc.gpsimd.dma_start(
                        out=gw_g[:, :, 512:1024],
                        in_=gwT[:, MLP + 512 * g:MLP + 512 * (g + 1)].rearrange(
                            "(a p) q -> p a q", p=128))
                    for t in range(8):
                        pg = pmm()
                        py = pmm()
                        for j in range(8):
                            nc.tensor.matmul(
                                pg, x2qT[:, j, 128 * t:128 * (t + 1)],
                                gw_g[:, j, 0:512],
                                start=(j == 0), stop=(j == 7))
                        for j in range(8):
                            nc.tensor.matmul(
                                py, x2qT[:, j, 128 * t:128 * (t + 1)],
                                gw_g[:, j, 512:1024],
                                start=(j == 0), stop=(j == 7))
                        sil = p6.tile([128, 512], F32, tag="sil")
                        nc.scalar.activation(out=sil, in_=pg, func=AF.Silu,
                                             scale=dqCg[:, t:t + 1])
                        h2c = p6.tile([128, 512], BF, tag="h2c", bufs=3)
                        nc.vector.scalar_tensor_tensor(
                            out=h2c, in0=py, scalar=dqCg[:, t:t + 1],
                            in1=sil, op0=AL.mult, op1=AL.mult)
                        nc.sync.dma_start(
                            out=h2_d[128 * t:128 * (t + 1),
                                     512 * g:512 * (g + 1)],
                            in_=h2c)
                        nc.vector.tensor_reduce(out=amDg[:, t, g:g + 1],
                                                in_=h2c, axis=AX.X, op=AL.max,
                                                apply_absolute_value=True)
                        sqd = p6.tile([128, 512], BF, tag="sqd")
                        nc.scalar.activation(out=sqd, in_=h2c, func=AF.Square,
                                             accum_out=ssDg[:, t, g:g + 1])
                amD = p6.tile([128, 8], F32, tag="amD", bufs=1)
                ssD = p6.tile([128, 8], F32, tag="ssD", bufs=1)
                nc.vector.tensor_reduce(out=amD, in_=amDg, axis=AX.X,
                                        op=AL.max)
                nc.vector.tensor_reduce(out=ssD, in_=ssDg, axis=AX.X,
                                        op=AL.add)
                quant_cols(p6, amD, ssD, MLP, q127D, dqDo, iw["down"],
                           [128, 8], "qd")

            # ====== phase 7: round h2 + down matmuls + final residual ====
            with tc.tile_pool(name="p7", bufs=2) as p7:
                for t in range(8):
                    h2t = p7.tile([128, 32, 128], BF, tag="h2t")
                    for q in range(4):
                        h2r = p7.tile([128, D], BF, tag="h2r", bufs=3)
                        nc.scalar.dma_start(
                            out=h2r,
                            in_=h2_d[128 * t:128 * (t + 1),
                                     1024 * q:1024 * (q + 1)])
                        kqd = round_bf16(p7, h2r, q127D[:, t:t + 1], "rd",
                                         bufs=2)
                        nc.sync.dma_start(out=h2t[:, 8 * q:8 * (q + 1), :],
                                          in_=kqd, transpose=True)
                    outt = p7.tile([128, D], F32, tag="outt")
                    for ck in range(0, D, 512):
                        pdn = pmm()
                        for j2 in range(32):
                            nc.tensor.matmul(pdn, h2t[:, j2, :],
                                             dw_sb[:, j2, ck:ck + 512],
                                             start=(j2 == 0), stop=(j2 == 31))
                        uv = p7.tile([128, 512], F32, tag="uv")
                        nc.vector.scalar_tensor_tensor(
                            out=uv, in0=pdn, scalar=dqDo[:, t:t + 1],
                            in1=B_g2[:, ck:ck + 512],
                            op0=AL.mult, op1=AL.mult)
                        nc.vector.tensor_tensor(out=outt[:, ck:ck + 512],
                                                in0=uv,
                                                in1=xnew[:, t, ck:ck + 512],
                                                op=AL.add)
                    nc.scalar.dma_start(out=out_sl[128 * t:128 * (t + 1), :],
                                        in_=outt)
            pDW.__exit__(None, None, None)

    nc.finalize()
    return nc


@functools.lru_cache(maxsize=2)
def _build_cached(iw_items):
    return _build(dict(iw_items))


def kernel(x, c, adaln_w, adaln_b, wi, wf, wg, gnorm_w, wo, gate_w, down_w):
    x = np.ascontiguousarray(np.asarray(x, dtype=np.float32))
    c = np.ascontiguousarray(np.asarray(c, dtype=np.float32))
    adaln_w = np.asarray(adaln_w, dtype=np.float32)
    adaln_b = np.asarray(adaln_b, dtype=np.float32)
    gnorm_w = np.asarray(gnorm_w, dtype=np.float32)

    mi, iwi = _quant_w(np.asarray(wi, dtype=np.float32))
    mf, iwf = _quant_w(np.asarray(wf, dtype=np.float32))
    mg, iwg = _quant_w(np.asarray(wg, dtype=np.float32))
    mo, iwo = _quant_w(np.asarray(wo, dtype=np.float32))
    mgate, iwgate = _quant_w(np.asarray(gate_w, dtype=np.float32))
    mdown, iwdown = _quant_w(np.asarray(down_w, dtype=np.float32))

    iw = {"i": float(iwi), "f": float(iwf), "g": float(iwg), "o": float(iwo),
          "gate": float(iwgate), "down": float(iwdown)}
    nc = _build_cached(tuple(sorted(iw.items())))

    wiT_h = np.ascontiguousarray(mi.T)
    wfT_h = np.ascontiguousarray(mf.T)
    wgT_h = np.ascontiguousarray(mg.T)
    woT_h = np.ascontiguousarray(mo.T)
    gwT_h = np.ascontiguousarray(mgate.T)
    dwT_h = np.ascontiguousarray(mdown.T)
    adwT = np.ascontiguousarray(adaln_w.T)          # [D, 6D]
    gnr_h = np.ascontiguousarray(np.tile(gnorm_w, NH)[None, :])
    c_cols_h = np.ascontiguousarray(
        c.T.reshape(8, 128, B).transpose(1, 0, 2))   # [128, 8, B]

    in_maps = []
    for core in range(N_CORES):
        b, half = core // 2, core % 2
        mask = np.zeros((N_CORES, 1), np.float32)
        if half == 1:
            mask[core - 1, 0] = 1.0
        bm = np.zeros((B, 1), np.float32)
        bm[b, 0] = 1.0
        in_maps.append({
            "x_sl": np.ascontiguousarray(x[b, half * TOK:(half + 1) * TOK, :]),
            "c_cols": c_cols_h,
            "adw_sl": np.ascontiguousarray(adwT[:, 768 * core:768 * (core + 1)]),
            "adb_sl": np.ascontiguousarray(
                adaln_b[None, 768 * core:768 * (core + 1)]),
            "mask8": mask,
            "bsel": np.ones((1, B), np.float32),
            "bmask": bm,
            "gnr": gnr_h,
            "wiT": wiT_h, "wfT": wfT_h, "wgT": wgT_h, "woT": woT_h,
            "gwT": gwT_h, "dwT": dwT_h,
        })

    res = run_bass_kernel_spmd(nc, in_maps, core_ids=list(range(N_CORES)))
    out = np.zeros((B, T, D), np.float32)
    for core in range(N_CORES):
        b, half = core // 2, core % 2
        out[b, half * TOK:(half + 1) * TOK, :] = res.results[core]["out_sl"]
    return out


# revision 37
# speedup vs baseline: 1.0210x; 1.0089x over previous
"""Trainium2 Bass kernel for nn_DiTBlock (HGRN-attention DiT block).

Sharding: 8 cores = 4 batches x 2 half-sequences (1024 tokens each).
All matmuls are exact-integer bf16 matmuls (activations quantized to
int8-range integers in bf16; ternary weights quantized on host).

v6 structure:
  - per-token-tile pipelining (LN/quant/round per 128-token tile)
  - activation transposes via DMA-XBAR (bf16); only the f32 h-transpose
    uses the PE
  - AllGathers overlapped (adaln AG under the LN-stats sweep, the scan
    carry AG under interleaved wg matmuls)
  - exact fp32 C_MAGIC rounding; subtract step on ScalarE
  - scan outputs (ha/cam) spilled to DRAM per chunk, reloaded for the
    carry fix; h2 spilled pre-round as bf16
  - weight loads on the scalar/gpsimd HWDGE queues so they never queue
    behind compute-dependent DMAs
"""
import functools
import numpy as np
import ml_dtypes

import concourse.bass as bass
import concourse.bacc as bacc_mod
import concourse.mybir as mybir
import concourse.tile as tile
from concourse.masks import make_identity
from concourse.bass_utils import run_bass_kernel_spmd

BF16 = ml_dtypes.bfloat16
F32 = mybir.dt.float32
BF = mybir.dt.bfloat16
U32 = mybir.dt.uint32
AL = mybir.AluOpType
AF = mybir.ActivationFunctionType
AX = mybir.AxisListType

B, T, D = 4, 2048, 1024
TOK = 1024          # tokens per core
NH, HD = 16, 64
MLP = 4096
N_CORES = 8
CMAGIC = float(1.5 * 2 ** 23)
MAGIC_U32 = 0x5F3759DF


def _quant_w(w):
    invws = float(np.clip(np.abs(w).mean(dtype=np.float64), 1e-5, None))
    m = np.clip(np.round(w.astype(np.float64) / invws), -1, 1).astype(np.float32)
    return np.ascontiguousarray(m.astype(BF16)), np.float32(invws)


def _build(iw):
    nc = bacc_mod.Bacc("TRN2", target_bir_lowering=False)

    x_sl = nc.declare_dram_parameter("x_sl", [TOK, D], F32, isOutput=False)
    c_cols = nc.declare_dram_parameter("c_cols", [128, 8, B], F32, isOutput=False)
    adw_sl = nc.declare_dram_parameter("adw_sl", [D, 768], F32, isOutput=False)
    adb_sl = nc.declare_dram_parameter("adb_sl", [1, 768], F32, isOutput=False)
    mask8 = nc.declare_dram_parameter("mask8", [N_CORES, 1], F32, isOutput=False)
    bsel = nc.declare_dram_parameter("bsel", [1, B], F32, isOutput=False)
    bmask = nc.declare_dram_parameter("bmask", [B, 1], F32, isOutput=False)
    gnr = nc.declare_dram_parameter("gnr", [1, D], F32, isOutput=False)
    wiT = nc.declare_dram_parameter("wiT", [D, D], BF, isOutput=False)
    wfT = nc.declare_dram_parameter("wfT", [D, D], BF, isOutput=False)
    wgT = nc.declare_dram_parameter("wgT", [D, D], BF, isOutput=False)
    woT = nc.declare_dram_parameter("woT", [D, D], BF, isOutput=False)
    gwT = nc.declare_dram_parameter("gwT", [D, 2 * MLP], BF, isOutput=False)
    dwT = nc.declare_dram_parameter("dwT", [MLP, D], BF, isOutput=False)
    out_sl = nc.declare_dram_parameter("out_sl", [TOK, D], F32, isOutput=True)

    cc1_in = nc.dram_tensor("cc1_in", [B, 768], F32)
    cc1_out = nc.dram_tensor("cc1_out", [N_CORES * B, 768], F32,
                             addr_space="Shared")
    cc2_in = nc.dram_tensor("cc2_in", [D], F32)
    cc2_out = nc.dram_tensor("cc2_out", [N_CORES, D], F32, addr_space="Shared")

    RG = [list(range(N_CORES))]

    with tile.TileContext(nc) as tc:
        import contextlib
        es = contextlib.ExitStack()
        with es:
            cst = es.enter_context(tc.tile_pool(name="cst", bufs=1))
            ps = es.enter_context(tc.tile_pool(name="ps", bufs=4, space="PSUM"))
            psm = es.enter_context(tc.tile_pool(name="psm", bufs=1, space="PSUM"))
            pst = es.enter_context(tc.tile_pool(name="pst", bufs=2, space="PSUM"))
            dr = es.enter_context(tc.tile_pool(name="dr", bufs=1, space="DRAM"))

            def pmm():
                return ps.tile([128, 512], F32, tag="mm", name="mm")

            def newton_rsqrt(sb, x_ap, scale, bias, shape, tag, iters=3):
                """rsqrt(x*scale + bias) via bit-magic + Newton iters."""
                t = sb.tile(shape, F32, tag=tag + "_t", name=tag + "_t")
                nc.vector.tensor_scalar(out=t, in0=x_ap, scalar1=float(scale),
                                        scalar2=float(bias), op0=AL.mult,
                                        op1=AL.add)
                y = sb.tile(shape, F32, tag=tag + "_y", name=tag + "_y")
                sh = sb.tile(shape, F32, tag=tag + "_s", name=tag + "_s")
                nc.vector.tensor_scalar(out=sh[:].bitcast(U32),
                                        in0=t[:].bitcast(U32), scalar1=1,
                                        scalar2=None,
                                        op0=AL.logical_shift_right)
                mg = sb.tile(shape, F32, tag=tag + "_m", name=tag + "_m")
                nc.vector.memset(mg[:].bitcast(U32), MAGIC_U32)
                nc.vector.tensor_tensor(out=y[:].bitcast(U32),
                                        in0=mg[:].bitcast(U32),
                                        in1=sh[:].bitcast(U32), op=AL.subtract)
                e = sb.tile(shape, F32, tag=tag + "_e", name=tag + "_e")
                for _ in range(iters):
                    nc.vector.tensor_tensor(out=e, in0=y, in1=y, op=AL.mult)
                    nc.vector.tensor_tensor(out=e, in0=e, in1=t, op=AL.mult)
                    nc.vector.tensor_scalar(out=e, in0=e, scalar1=-0.5,
                                            scalar2=1.5, op0=AL.mult,
                                            op1=AL.add)
                    nc.vector.tensor_tensor(out=y, in0=y, in1=e, op=AL.mult)
                return y

            def quant_cols(sb, am_col, ss_col, dk, q_col, dq_col, dq_scale,
                           shape, tag):
                """q = 127/max(am,1e-5); dq = am*rsqrt(ss/dk+1e-8)*scale/127"""
                amc = sb.tile(shape, F32, tag=tag + "amc", name=tag + "amc")
                nc.vector.tensor_scalar(out=amc, in0=am_col, scalar1=1e-5,
                                        scalar2=None, op0=AL.max)
                rec = sb.tile(shape, F32, tag=tag + "rec", name=tag + "rec")
                nc.vector.reciprocal(out=rec, in_=amc)
                nc.vector.tensor_scalar(out=q_col, in0=rec, scalar1=127.0,
                                        scalar2=None, op0=AL.mult)
                rs = newton_rsqrt(sb, ss_col, 1.0 / dk, 1e-8, shape,
                                  tag + "rs", iters=2)
                nc.vector.tensor_tensor(out=dq_col, in0=amc, in1=rs,
                                        op=AL.mult)
                nc.vector.tensor_scalar(out=dq_col, in0=dq_col,
                                        scalar1=float(dq_scale) / 127.0,
                                        scalar2=None, op0=AL.mult)

            # ---------------- consts (whole-kernel lifetime) -------------
            identf = cst.tile([128, 128], F32)
            make_identity(nc, identf)
            ones_row = cst.tile([1, 128], F32)
            nc.vector.memset(ones_row, 1.0)
            negmagic = cst.tile([128, 1], F32)
            nc.vector.memset(negmagic, -CMAGIC)
            mask_sb = cst.tile([N_CORES, 1], F32)
            nc.sync.dma_start(out=mask_sb, in_=mask8[:, :])
            bsel_sb = cst.tile([1, B], F32)
            nc.sync.dma_start(out=bsel_sb, in_=bsel[:, :])
            bmask_sb = cst.tile([B, 1], F32)
            nc.sync.dma_start(out=bmask_sb, in_=bmask[:, :])

            def round_bf16(sb, src, q_col, tagp, bufs=3):
                """round(src*q) -> integer-valued bf16 tile via fp32 magic."""
                t2 = sb.tile([128, src.free_size()], F32, tag=tagp + "t2",
                             name=tagp + "t2")
                nc.vector.tensor_scalar(out=t2, in0=src, scalar1=q_col,
                                        scalar2=CMAGIC, op0=AL.mult,
                                        op1=AL.add)
                kq = sb.tile([128, src.free_size()], BF, tag=tagp + "kq",
                             name=tagp + "kq", bufs=bufs)
                nc.scalar.activation(out=kq, in_=t2, func=AF.Identity,
                                     bias=negmagic)
                return kq

            q127A = cst.tile([128, 8], F32); dqA = cst.tile([128, 8], F32)
            dqAg = cst.tile([128, 8], F32)
            q127O = cst.tile([128, 8], F32); dqOo = cst.tile([128, 8], F32)
            q127C = cst.tile([128, 8], F32); dqCg = cst.tile([128, 8], F32)
            q127D = cst.tile([128, 8], F32); dqDo = cst.tile([128, 8], F32)
            B_gn = cst.tile([128, D], F32)
            B_g1 = cst.tile([128, D], F32)
            B_sh2 = cst.tile([128, D], F32)
            B_sc2 = cst.tile([128, D], F32)
            B_g2 = cst.tile([128, D], F32)

            params_d = dr.tile([6 * D], F32, tag="params")
            ha_d = dr.tile([TOK, D], F32, tag="had")
            cam_d = dr.tile([TOK, D], BF, tag="camd")
            h2_d = dr.tile([TOK, MLP], BF, tag="h2d")

            # whole-kernel SBUF: xqT (-> x2qT) and xnew
            big = es.enter_context(tc.tile_pool(name="big", bufs=1))
            xqT = big.tile([128, 8, TOK], BF, tag="xq", bufs=1)

            def bcast_row(row_ap, dst, plus1=False):
                for ch in range(0, D, 512):
                    pb = pst.tile([128, 512], F32, tag="aux", name="aux")
                    nc.tensor.matmul(pb, ones_row, row_ap[:, ch:ch + 512],
                                     start=True, stop=True)
                    if plus1:
                        nc.scalar.activation(out=dst[:, ch:ch + 512], in_=pb,
                                             func=AF.Identity, bias=1.0)
                    else:
                        nc.scalar.copy(out=dst[:, ch:ch + 512], in_=pb)

            # =========== phase 0: adaln (+AG) and LN stats ===========
            with tc.tile_pool(name="p0", bufs=2) as p0:
                B_sh1 = p0.tile([128, D], F32, tag="Bsh1", bufs=1)
                B_sc1 = p0.tile([128, D], F32, tag="Bsc1", bufs=1)
                gnr_sb = p0.tile([1, D], F32, tag="gnr", bufs=1)
                nc.sync.dma_start(out=gnr_sb, in_=gnr[:, :])

                # adaln: this core computes 768 of the 6144 outputs
                c_sb = p0.tile([128, 8, B], F32, tag="csb", bufs=1)
                nc.sync.dma_start(out=c_sb, in_=c_cols[:, :, :])
                cs_sb = p0.tile([128, 8, B], F32, tag="cssb", bufs=1)
                nc.scalar.activation(out=cs_sb, in_=c_sb, func=AF.Silu)
                adb_sb = p0.tile([1, 768], F32, tag="adb", bufs=1)
                nc.sync.dma_start(out=adb_sb, in_=adb_sl[:, :])
                psA = psm.tile([B, 512], F32, tag="sm")
                psB = psm.tile([B, 256], F32, tag="sm2")
                for j in range(8):
                    adw_j = p0.tile([128, 768], F32, tag="adw")
                    nc.sync.dma_start(out=adw_j,
                                      in_=adw_sl[128 * j:128 * (j + 1), :])
                    nc.tensor.matmul(psA, cs_sb[:, j, :], adw_j[:, 0:512],
                                     start=(j == 0), stop=False)
                    nc.tensor.matmul(psB, cs_sb[:, j, :], adw_j[:, 512:768],
                                     start=(j == 0), stop=False)
                nc.tensor.matmul(psA, bsel_sb, adb_sb[:, 0:512],
                                 start=False, stop=True)
                nc.tensor.matmul(psB, bsel_sb, adb_sb[:, 512:768],
                                 start=False, stop=True)
                ad_sb = p0.tile([B, 768], F32, tag="adsb", bufs=1)
                nc.scalar.copy(out=ad_sb[:, 0:512], in_=psA)
                nc.scalar.copy(out=ad_sb[:, 512:768], in_=psB)
                nc.sync.dma_start(out=cc1_in[:, :], in_=ad_sb)
                nc.gpsimd.collective_compute(
                    "AllGather", AL.bypass, ins=[cc1_in[:]],
                    outs=[cc1_out[:]], replica_groups=RG)

                # LN stats pass (params-independent -> overlaps the AG)
                amA = p0.tile([128, 8], F32, tag="amA", bufs=1)
                ssA = p0.tile([128, 8], F32, tag="ssA", bufs=1)
                rstdA = p0.tile([128, 8], F32, tag="rstdA", bufs=1)
                nmrA = p0.tile([128, 8], F32, tag="nmrA", bufs=1)
                for i in range(8):
                    xt = p0.tile([128, D], F32, tag="xt", bufs=2)
                    nc.sync.dma_start(out=xt,
                                      in_=x_sl[128 * i:128 * (i + 1), :])
                    st = p0.tile([128, 2, 6], F32, tag="bst")
                    xr = xt.rearrange("p (s d) -> p s d", s=2)
                    for s2 in range(2):
                        nc.vector.bn_stats(out=st[:, s2, :], in_=xr[:, s2, :])
                    mv = p0.tile([128, 2], F32, tag="bmv")
                    nc.vector.bn_aggr(out=mv, in_=st)
                    rstdLN = newton_rsqrt(p0, mv[:, 1:2], 1.0, 1e-6,
                                          [128, 1], "rLN")
                    nc.vector.tensor_copy(out=rstdA[:, i:i + 1], in_=rstdLN)
                    nc.vector.tensor_tensor(out=nmrA[:, i:i + 1],
                                            in0=mv[:, 0:1], in1=rstdLN,
                                            op=AL.mult)
                nc.vector.tensor_scalar(out=nmrA, in0=nmrA, scalar1=-1.0,
                                        scalar2=None, op0=AL.mult)

                # select this batch's row per 768-block, stage through DRAM
                for r in range(8):
                    ag_r = p0.tile([B, 768], F32, tag="ag1")
                    nc.sync.dma_start(out=ag_r,
                                      in_=cc1_out[4 * r:4 * (r + 1), :])
                    pp1 = psm.tile([1, 512], F32, tag="sm")
                    pp2 = psm.tile([1, 256], F32, tag="sm2")
                    nc.tensor.matmul(pp1, bmask_sb, ag_r[:, 0:512],
                                     start=True, stop=True)
                    nc.tensor.matmul(pp2, bmask_sb, ag_r[:, 512:768],
                                     start=True, stop=True)
                    rb7 = p0.tile([1, 768], F32, tag="rb7", bufs=2)
                    nc.scalar.copy(out=rb7[:, 0:512], in_=pp1)
                    nc.scalar.copy(out=rb7[:, 512:768], in_=pp2)
                    nc.sync.dma_start(
                        out=params_d[768 * r:768 * (r + 1)].rearrange(
                            "(one c) -> one c", one=1),
                        in_=rb7)
                B_list = [(B_sh1, False), (B_sc1, True), (B_g1, False),
                          (B_sh2, False), (B_sc2, True), (B_g2, False)]
                for k, (dst, plus1) in enumerate(B_list):
                    rbk = p0.tile([1, D], F32, tag="rbk", bufs=2)
                    nc.sync.dma_start(
                        out=rbk,
                        in_=params_d[D * k:D * (k + 1)].rearrange(
                            "(one c) -> one c", one=1))
                    bcast_row(rbk, dst, plus1=plus1)
                bcast_row(gnr_sb, B_gn)

                # ===== phase 1: modulate + quant + round (per tile) =====
                for i in range(8):
                    xt = p0.tile([128, D], F32, tag="xt2", bufs=2)
                    nc.sync.dma_start(out=xt,
                                      in_=x_sl[128 * i:128 * (i + 1), :])
                    u = p0.tile([128, D], F32, tag="u")
                    nc.scalar.activation(out=u, in_=xt, func=AF.Identity,
                                         scale=rstdA[:, i:i + 1],
                                         bias=nmrA[:, i:i + 1])
                    ttm = p0.tile([128, D], F32, tag="ttm")
                    nc.vector.tensor_tensor(out=ttm, in0=u, in1=B_sc1,
                                            op=AL.mult)
                    moda = p0.tile([128, D], F32, tag="moda", bufs=2)
                    nc.vector.tensor_tensor(out=moda, in0=ttm, in1=B_sh1,
                                            op=AL.add)
                    nc.vector.tensor_reduce(out=amA[:, i:i + 1], in_=moda,
                                            axis=AX.X, op=AL.max,
                                            apply_absolute_value=True)
                    sqs = p0.tile([128, D], BF, tag="sqs")
                    nc.scalar.activation(out=sqs, in_=moda, func=AF.Square,
                                         accum_out=ssA[:, i:i + 1])
                    quant_cols(p0, amA[:, i:i + 1], ssA[:, i:i + 1], D,
                               q127A[:, i:i + 1], dqA[:, i:i + 1], 1.0,
                               [128, 1], "qa")
                    nc.vector.tensor_scalar(out=dqAg[:, i:i + 1],
                                            in0=dqA[:, i:i + 1],
                                            scalar1=float(iw["g"]),
                                            scalar2=None, op0=AL.mult)
                    kq = round_bf16(p0, moda, q127A[:, i:i + 1], "ra")
                    nc.sync.dma_start(out=xqT[:, :, 128 * i:128 * (i + 1)],
                                      in_=kq, transpose=True)

            # ====== phase 2: i/f + g matmuls + scan (chunked) ======
            pG = tc.tile_pool(name="pG", bufs=1)
            pgs = pG.__enter__()
            gs = pgs.tile([128, 8, D], F32, tag="gs", bufs=1)
            with tc.tile_pool(name="p2", bufs=2) as p2:
                # Sb_i / Sb_f: dq row broadcast over partitions, scaled by iw
                Sb_i = p2.tile([128, TOK], F32, tag="Sbi", bufs=1)
                Sb_f = p2.tile([128, TOK], F32, tag="Sbf", bufs=1)
                dqrow_sb = p2.tile([1, D], F32, tag="dqrow", bufs=1)
                for i8 in range(8):
                    ptr = psm.tile([1, 128], F32, tag="sm")
                    nc.tensor.transpose(ptr, dqA[:, i8:i8 + 1], identf)
                    nc.scalar.copy(out=dqrow_sb[:, 128 * i8:128 * (i8 + 1)],
                                   in_=ptr)
                oi = p2.tile([1, 128], F32, tag="oi", bufs=1)
                nc.vector.memset(oi, float(iw["i"]))
                of = p2.tile([1, 128], F32, tag="of", bufs=1)
                nc.vector.memset(of, float(iw["f"]))
                for ch in range(0, TOK, 512):
                    pb = pst.tile([128, 512], F32, tag="aux", name="aux")
                    nc.tensor.matmul(pb, oi, dqrow_sb[:, ch:ch + 512],
                                     start=True, stop=True)
                    nc.scalar.copy(out=Sb_i[:, ch:ch + 512], in_=pb)
                    pb2 = pst.tile([128, 512], F32, tag="aux", name="aux")
                    nc.tensor.matmul(pb2, of, dqrow_sb[:, ch:ch + 512],
                                     start=True, stop=True)
                    nc.scalar.copy(out=Sb_f[:, ch:ch + 512], in_=pb2)

                wf_sb = p2.tile([128, 8, D], BF, tag="wfsb", bufs=1)
                nc.scalar.dma_start(
                    out=wf_sb,
                    in_=wfT[:, :].rearrange("(a p) q -> p a q", p=128))
                wi_sb = p2.tile([128, 8, D], BF, tag="wisb", bufs=1)
                nc.scalar.dma_start(
                    out=wi_sb,
                    in_=wiT[:, :].rearrange("(a p) q -> p a q", p=128))
                wg_sb = p2.tile([128, 8, D], BF, tag="wgsb", bufs=1)
                nc.scalar.dma_start(
                    out=wg_sb,
                    in_=wgT[:, :].rearrange("(a p) q -> p a q", p=128))

                def g_mms(trange):
                    for t in trange:
                        for ck2 in range(0, D, 512):
                            pg = pmm()
                            for j in range(8):
                                nc.tensor.matmul(
                                    pg, xqT[:, j, 128 * t:128 * (t + 1)],
                                    wg_sb[:, j, ck2:ck2 + 512],
                                    start=(j == 0), stop=(j == 7))
                            scr = p2.tile([128, 512], F32, tag="gscr",
                                          bufs=2)
                            nc.scalar.activation(out=scr, in_=pg,
                                                 func=AF.Silu,
                                                 scale=dqAg[:, t:t + 1])
                            nc.vector.tensor_tensor(
                                out=gs[:, t, ck2:ck2 + 512], in0=scr,
                                in1=B_gn[:, ck2:ck2 + 512], op=AL.mult)

                ha_last = p2.tile([128, 8], F32, tag="halast", bufs=1)
                cam_last = p2.tile([128, 8], F32, tag="camlast", bufs=1)

                def scan_chain(ft, it, m, ck, cki):
                    sigf = p2.tile([128, 512], F32, tag="sigf", bufs=2)
                    nc.scalar.activation(out=sigf, in_=ft, func=AF.Sigmoid)
                    omf = p2.tile([128, 512], F32, tag="omf", bufs=2)
                    nc.scalar.activation(out=omf, in_=ft, func=AF.Sigmoid,
                                         scale=-1.0)
                    sili = p2.tile([128, 512], F32, tag="sili", bufs=2)
                    nc.scalar.activation(out=sili, in_=it, func=AF.Silu)
                    ifin = p2.tile([128, 512], F32, tag="ifin", bufs=2)
                    nc.vector.tensor_tensor(out=ifin, in0=sili, in1=omf,
                                            op=AL.mult)
                    ha_c = p2.tile([128, 512], F32, tag="hac", bufs=3)
                    init_h = 0.0 if cki == 0 else ha_last[:, m:m + 1]
                    nc.vector.tensor_tensor_scan(
                        ha_c, sigf, ifin, init_h, op0=AL.mult, op1=AL.add)
                    cam_c = p2.tile([128, 512], BF, tag="camc", bufs=3)
                    init_c = 1.0 if cki == 0 else cam_last[:, m:m + 1]
                    nc.vector.tensor_tensor_scan(
                        cam_c, sigf, sigf, init_c, op0=AL.mult, op1=AL.bypass)
                    nc.sync.dma_start(
                        out=ha_d[128 * m:128 * (m + 1), ck:ck + 512],
                        in_=ha_c)
                    nc.sync.dma_start(
                        out=cam_d[128 * m:128 * (m + 1), ck:ck + 512],
                        in_=cam_c)
                    if cki == 0:
                        nc.vector.tensor_copy(out=ha_last[:, m:m + 1],
                                              in_=ha_c[:, 511:512])
                        nc.vector.tensor_copy(out=cam_last[:, m:m + 1],
                                              in_=cam_c[:, 511:512])
                    else:
                        nc.sync.dma_start(
                            out=cc2_in[128 * m:128 * (m + 1)].rearrange(
                                "(p one) -> p one", one=1),
                            in_=ha_c[:, 511:512])

                pending = None
                for cki, ck in enumerate(range(0, TOK, 512)):
                    for m in range(8):
                        pf = pmm()
                        pi = pmm()
                        for j in range(8):
                            nc.tensor.matmul(
                                pf, wf_sb[:, j, 128 * m:128 * (m + 1)],
                                xqT[:, j, ck:ck + 512],
                                start=(j == 0), stop=(j == 7))
                        for j in range(8):
                            nc.tensor.matmul(
                                pi, wi_sb[:, j, 128 * m:128 * (m + 1)],
                                xqT[:, j, ck:ck + 512],
                                start=(j == 0), stop=(j == 7))
                        ft = p2.tile([128, 512], F32, tag="ftm", bufs=3)
                        nc.vector.tensor_tensor(out=ft, in0=pf,
                                                in1=Sb_f[:, ck:ck + 512],
                                                op=AL.mult)
                        it = p2.tile([128, 512], F32, tag="itm", bufs=3)
                        nc.vector.tensor_tensor(out=it, in0=pi,
                                                in1=Sb_i[:, ck:ck + 512],
                                                op=AL.mult)
                        if pending is not None:
                            scan_chain(*pending)
                        pending = (ft, it, m, ck, cki)
                    if cki == 1:
                        scan_chain(*pending)
                        pending = None
                    g_mms(range(0, 4) if cki == 0 else range(4, 8))
                nc.gpsimd.collective_compute(
                    "AllGather", AL.bypass, ins=[cc2_in[:]], outs=[cc2_out[:]],
                    replica_groups=RG)

            # =========== phase 3: carry fix + hT (PE transpose) ==========
            pO = tc.tile_pool(name="pO", bufs=1)
            pos_ = pO.__enter__()
            oqT = pos_.tile([128, 8, D], BF, tag="oqT", bufs=1)
            wo_sb = pos_.tile([128, 8, D], BF, tag="wosb", bufs=1)
            nc.scalar.dma_start(
                out=wo_sb,
                in_=woT[:, :].rearrange("(a p) q -> p a q", p=128))
            pH3 = tc.tile_pool(name="pH3", bufs=1)
            ph3_ = pH3.__enter__()
            hT = ph3_.tile([128, 8, D], F32, tag="hT", bufs=1)
            with tc.tile_pool(name="p3", bufs=2) as p3:
                ag2 = p3.tile([N_CORES, D], F32, tag="ag2", bufs=1)
                nc.sync.dma_start(out=ag2, in_=cc2_out[:, :])
                for m in range(8):
                    pc = psm.tile([128, 1], F32, tag="sm")
                    nc.tensor.matmul(pc, ag2[:, 128 * m:128 * (m + 1)],
                                     mask_sb, start=True, stop=True)
                    carry = p3.tile([128, 1], F32, tag="carry")
                    nc.scalar.copy(out=carry, in_=pc)
                    har = p3.tile([128, TOK], F32, tag="har", bufs=2)
                    nc.sync.dma_start(out=har,
                                      in_=ha_d[128 * m:128 * (m + 1), :])
                    camr = p3.tile([128, TOK], BF, tag="camr", bufs=2)
                    nc.sync.dma_start(out=camr,
                                      in_=cam_d[128 * m:128 * (m + 1), :])
                    hfix = p3.tile([128, TOK], F32, tag="hfix", bufs=2)
                    nc.vector.scalar_tensor_tensor(out=hfix, in0=camr,
                                                   scalar=carry, in1=har,
                                                   op0=AL.mult, op1=AL.add)
                    for g4 in range(0, 8, 4):
                        tp = pst.tile([128, 512], F32, tag="aux")
                        for jj in range(4):
                            t_i = g4 + jj
                            nc.tensor.transpose(
                                tp[:, 128 * jj:128 * (jj + 1)],
                                hfix[:, 128 * t_i:128 * (t_i + 1)], identf)
                        for jj in range(4):
                            t_i = g4 + jj
                            if jj % 2 == 0:
                                nc.scalar.copy(
                                    out=hT[:, t_i, 128 * m:128 * (m + 1)],
                                    in_=tp[:, 128 * jj:128 * (jj + 1)])
                            else:
                                nc.vector.tensor_copy(
                                    out=hT[:, t_i, 128 * m:128 * (m + 1)],
                                    in_=tp[:, 128 * jj:128 * (jj + 1)])

            # =========== phase 4: gnorm-swish-gate + o quant ===========
            with tc.tile_pool(name="p4", bufs=2) as p4:
                amO = p4.tile([128, 8], F32, tag="amO", bufs=1)
                ssO = p4.tile([128, 8], F32, tag="ssO", bufs=1)
                for t in range(8):
                    sq = p4.tile([128, D], F32, tag="sq")
                    nc.scalar.activation(out=sq, in_=hT[:, t, :],
                                         func=AF.Square)
                    msh = p4.tile([128, 16], F32, tag="msh")
                    nc.vector.tensor_reduce(
                        out=msh,
                        in_=sq.rearrange("p (h d) -> p h d", h=NH),
                        axis=AX.X, op=AL.add)
                    rstdH = newton_rsqrt(p4, msh, 1.0 / HD, 1e-5, [128, 16],
                                         "rH")
                    hn = p4.tile([128, D], F32, tag="hn")
                    rb = bass.AP(tensor=rstdH.tensor, offset=rstdH.offset,
                                 ap=[rstdH.ap[0], [1, NH], [0, HD]])
                    nc.vector.tensor_tensor(
                        out=hn.rearrange("p (h d) -> p h d", h=NH),
                        in0=hT[:, t, :].rearrange("p (h d) -> p h d", h=NH),
                        in1=rb, op=AL.mult)
                    oa = p4.tile([128, D], F32, tag="oa", bufs=2)
                    nc.vector.tensor_tensor(out=oa, in0=hn, in1=gs[:, t, :],
                                            op=AL.mult)
                    nc.vector.tensor_reduce(out=amO[:, t:t + 1], in_=oa,
                                            axis=AX.X, op=AL.max,
                                            apply_absolute_value=True)
                    sqo = p4.tile([128, D], BF, tag="sqo", bufs=1)
                    nc.scalar.activation(out=sqo, in_=oa, func=AF.Square,
                                         accum_out=ssO[:, t:t + 1])
                    quant_cols(p4, amO[:, t:t + 1], ssO[:, t:t + 1], D,
                               q127O[:, t:t + 1], dqOo[:, t:t + 1], iw["o"],
                               [128, 1], "qo")
                    kq = round_bf16(p4, oa, q127O[:, t:t + 1], "ro")
                    nc.sync.dma_start(out=oqT[:, :, 128 * t:128 * (t + 1)],
                                      in_=kq, transpose=True)
            pH3.__exit__(None, None, None)  # free hT

            # ====== phase 5: wo matmul + residual + LN2 + quant ======
            xnew = big.tile([128, 8, D], F32, tag="xnew", bufs=1)
            x2qT = big.tile([128, 8, TOK], BF, tag="xq", bufs=1)
            with tc.tile_pool(name="p5", bufs=2) as p5:
                amC = p5.tile([128, 8], F32, tag="amC", bufs=1)
                ssC = p5.tile([128, 8], F32, tag="ssC", bufs=1)
                for t in range(8):
                    xr2 = p5.tile([128, D], F32, tag="xr2", bufs=2)
                    nc.sync.dma_start(out=xr2,
                                      in_=x_sl[128 * t:128 * (t + 1), :])
                    xn = xnew[:, t, :]
                    for ck in range(0, D, 512):
                        pw = pmm()
                        for j in range(8):
                            nc.tensor.matmul(
                                pw, oqT[:, j, 128 * t:128 * (t + 1)],
                                wo_sb[:, j, ck:ck + 512],
                                start=(j == 0), stop=(j == 7))
                        ug = p5.tile([128, 512], F32, tag="ug")
                        nc.vector.scalar_tensor_tensor(
                            out=ug, in0=pw, scalar=dqOo[:, t:t + 1],
                            in1=B_g1[:, ck:ck + 512],
                            op0=AL.mult, op1=AL.mult)
                        nc.vector.tensor_tensor(out=xn[:, ck:ck + 512],
                                                in0=ug,
                                                in1=xr2[:, ck:ck + 512],
                                                op=AL.add)
                    st = p5.tile([128, 2, 6], F32, tag="bst2")
                    xrr = xn.rearrange("p (s d) -> p s d", s=2)
                    for s2 in range(2):
                        nc.vector.bn_stats(out=st[:, s2, :], in_=xrr[:, s2, :])
                    mv = p5.tile([128, 2], F32, tag="bmv2")
                    nc.vector.bn_aggr(out=mv, in_=st)
                    rstdC = newton_rsqrt(p5, mv[:, 1:2], 1.0, 1e-6, [128, 1],
                                         "rC")
                    nmrC = p5.tile([128, 1], F32, tag="nmrC")
                    nc.vector.tensor_tensor(out=nmrC, in0=mv[:, 0:1],
                                            in1=rstdC, op=AL.mult)
                    nc.vector.tensor_scalar(out=nmrC, in0=nmrC, scalar1=-1.0,
                                            scalar2=None, op0=AL.mult)
                    u2 = p5.tile([128, D], F32, tag="u2")
                    nc.scalar.activation(out=u2, in_=xn, func=AF.Identity,
                                         scale=rstdC, bias=nmrC)
                    tt2 = p5.tile([128, D], F32, tag="tt2")
                    nc.vector.tensor_tensor(out=tt2, in0=u2, in1=B_sc2,
                                            op=AL.mult)
                    mod2 = p5.tile([128, D], F32, tag="mod2", bufs=2)
                    nc.vector.tensor_tensor(out=mod2, in0=tt2, in1=B_sh2,
                                            op=AL.add)
                    nc.vector.tensor_reduce(out=amC[:, t:t + 1], in_=mod2,
                                            axis=AX.X, op=AL.max,
                                            apply_absolute_value=True)
                    sqc = p5.tile([128, D], BF, tag="sqc")
                    nc.scalar.activation(out=sqc, in_=mod2, func=AF.Square,
                                         accum_out=ssC[:, t:t + 1])
                    quant_cols(p5, amC[:, t:t + 1], ssC[:, t:t + 1], D,
                               q127C[:, t:t + 1], dqCg[:, t:t + 1], iw["gate"],
                               [128, 1], "qc")
                    kq = round_bf16(p5, mod2, q127C[:, t:t + 1], "rc")
                    nc.sync.dma_start(out=x2qT[:, :, 128 * t:128 * (t + 1)],
                                      in_=kq, transpose=True)
            pO.__exit__(None, None, None)   # free oqT/wo
            pG.__exit__(None, None, None)   # free gs

            # =========== phase 6: MLP gate matmuls -> h2 (bf16) ==========
            pDW = tc.tile_pool(name="pDW", bufs=1)
            pdw = pDW.__enter__()
            dw_sb = pdw.tile([128, 32, D], BF, tag="dwsb", bufs=1)
            nc.gpsimd.dma_start(
                out=dw_sb,
                in_=dwT[:, :].rearrange("(a p) q -> p a q", p=128))
            with tc.tile_pool(name="p6", bufs=2) as p6:
                amDg = p6.tile([128, 8, 8], F32, tag="amDg", bufs=1)
                ssDg = p6.tile([128, 8, 8], F32, tag="ssDg", bufs=1)
                for g in range(8):
                    gw_g = p6.tile([128, 8, 1024], BF, tag="gwg")
                    nc.gpsimd.dma_start(
                        out=gw_g[:, :, 0:512],
                        in_=gwT[:, 512 * g:512 * (g + 1)].rearrange(
                            "(a p) q -> p a q", p=128))
                    nc.gpsimd.dma_start(
                        out=gw_g[:, :, 512:1024],
                        in_=gwT[:, MLP + 512 * g:MLP + 512 * (g + 1)].rearrange(
                            "(a p) q -> p a q", p=128))
                    for t in range(8):
                        pg = pmm()
                        py = pmm()
                        for j in range(8):
                            nc.tensor.matmul(
                                pg, x2qT[:, j, 128 * t:128 * (t + 1)],
                                gw_g[:, j, 0:512],
                                start=(j == 0), stop=(j == 7))
                        for j in range(8):
                            nc.tensor.matmul(
                                py, x2qT[:, j, 128 * t:128 * (t + 1)],
                                gw_g[:, j, 512:1024],
                                start=(j == 0), stop=(j == 7))
                        sil = p6.tile([128, 512], F32, tag="sil")
                        nc.scalar.activation(out=sil, in_=pg, func=AF.Silu,
                                             scale=dqCg[:, t:t + 1])
                        h2c = p6.tile([128, 512], BF, tag="h2c", bufs=3)
                        nc.vector.scalar_tensor_tensor(
                            out=h2c, in0=py, scalar=dqCg[:, t:t + 1],
                            in1=sil, op0=AL.mult, op1=AL.mult)
                        nc.sync.dma_start(
                            out=h2_d[128 * t:128 * (t + 1),
                                     512 * g:512 * (g + 1)],
                            in_=h2c)
                        nc.vector.tensor_reduce(out=amDg[:, t, g:g + 1],
                                                in_=h2c, axis=AX.X, op=AL.max,
                                                apply_absolute_value=True)
                        sqd = p6.tile([128, 512], BF, tag="sqd")
                        nc.scalar.activation(out=sqd, in_=h2c, func=AF.Square,
                                             accum_out=ssDg[:, t, g:g + 1])
                amD = p6.tile([128, 8], F32, tag="amD", bufs=1)
                ssD = p6.tile([128, 8], F32, tag="ssD", bufs=1)
                nc.vector.tensor_reduce(out=amD, in_=amDg, axis=AX.X,
                                        op=AL.max)
                nc.vector.tensor_reduce(out=ssD, in_=ssDg, axis=AX.X,
                                        op=AL.add)
                quant_cols(p6, amD, ssD, MLP, q127D, dqDo, iw["down"],
                           [128, 8], "qd")

            # ====== phase 7: round h2 + down matmuls + final residual ====
            with tc.tile_pool(name="p7", bufs=2) as p7:
                for t in range(8):
                    h2t = p7.tile([128, 32, 128], BF, tag="h2t")
                    for q in range(4):
                        h2r = p7.tile([128, D], BF, tag="h2r", bufs=3)
                        nc.scalar.dma_start(
                            out=h2r,
                            in_=h2_d[128 * t:128 * (t + 1),
                                     1024 * q:1024 * (q + 1)])
                        kqd = round_bf16(p7, h2r, q127D[:, t:t + 1], "rd",
                                         bufs=2)
                        nc.sync.dma_start(out=h2t[:, 8 * q:8 * (q + 1), :],
                                          in_=kqd, transpose=True)
                    outt = p7.tile([128, D], F32, tag="outt")
                    for ck in range(0, D, 512):
                        pdn = pmm()
                        for j2 in range(32):
                            nc.tensor.matmul(pdn, h2t[:, j2, :],
                                             dw_sb[:, j2, ck:ck + 512],
                                             start=(j2 == 0), stop=(j2 == 31))
                        uv = p7.tile([128, 512], F32, tag="uv")
                        nc.vector.scalar_tensor_tensor(
                            out=uv, in0=pdn, scalar=dqDo[:, t:t + 1],
                            in1=B_g2[:, ck:ck + 512],
                            op0=AL.mult, op1=AL.mult)
                        nc.vector.tensor_tensor(out=outt[:, ck:ck + 512],
                                                in0=uv,
                                                in1=xnew[:, t, ck:ck + 512],
                                                op=AL.add)
                    nc.scalar.dma_start(out=out_sl[128 * t:128 * (t + 1), :],
                                        in_=outt)
            pDW.__exit__(None, None, None)

    nc.finalize()
    return nc


@functools.lru_cache(maxsize=2)
def _build_cached(iw_items):
    return _build(dict(iw_items))


def kernel(x, c, adaln_w, adaln_b, wi, wf, wg, gnorm_w, wo, gate_w, down_w):
    x = np.ascontiguousarray(np.asarray(x, dtype=np.float32))
    c = np.ascontiguousarray(np.asarray(c, dtype=np.float32))
    adaln_w = np.asarray(adaln_w, dtype=np.float32)
    adaln_b = np.asarray(adaln_b, dtype=np.float32)
    gnorm_w = np.asarray(gnorm_w, dtype=np.float32)

    mi, iwi = _quant_w(np.asarray(wi, dtype=np.float32))
    mf, iwf = _quant_w(np.asarray(wf, dtype=np.float32))
    mg, iwg = _quant_w(np.asarray(wg, dtype=np.float32))
    mo, iwo = _quant_w(np.asarray(wo, dtype=np.float32))
    mgate, iwgate = _quant_w(np.asarray(gate_w, dtype=np.float32))
    mdown, iwdown = _quant_w(np.asarray(down_w, dtype=np.float32))

    iw = {"i": float(iwi), "f": float(iwf), "g": float(iwg), "o": float(iwo),
          "gate": float(iwgate), "down": float(iwdown)}
    nc = _build_cached(tuple(sorted(iw.items())))

    wiT_h = np.ascontiguousarray(mi.T)
    wfT_h = np.ascontiguousarray(mf.T)
    wgT_h = np.ascontiguousarray(mg.T)
    woT_h = np.ascontiguousarray(mo.T)
    gwT_h = np.ascontiguousarray(mgate.T)
    dwT_h = np.ascontiguousarray(mdown.T)
    adwT = np.ascontiguousarray(adaln_w.T)          # [D, 6D]
    gnr_h = np.ascontiguousarray(np.tile(gnorm_w, NH)[None, :])
    c_cols_h = np.ascontiguousarray(
        c.T.reshape(8, 128, B).transpose(1, 0, 2))   # [128, 8, B]

    in_maps = []
    for core in range(N_CORES):
        b, half = core // 2, core % 2
        mask = np.zeros((N_CORES, 1), np.float32)
        if half == 1:
            mask[core - 1, 0] = 1.0
        bm = np.zeros((B, 1), np.float32)
        bm[b, 0] = 1.0
        in_maps.append({
            "x_sl": np.ascontiguousarray(x[b, half * TOK:(half + 1) * TOK, :]),
            "c_cols": c_cols_h,
            "adw_sl": np.ascontiguousarray(adwT[:, 768 * core:768 * (core + 1)]),
            "adb_sl": np.ascontiguousarray(
                adaln_b[None, 768 * core:768 * (core + 1)]),
            "mask8": mask,
            "bsel": np.ones((1, B), np.float32),
            "bmask": bm,
            "gnr": gnr_h,
            "wiT": wiT_h, "wfT": wfT_h, "wgT": wgT_h, "woT": woT_h,
            "gwT": gwT_h, "dwT": dwT_h,
        })

    res = run_bass_kernel_spmd(nc, in_maps, core_ids=list(range(N_CORES)))
    out = np.zeros((B, T, D), np.float32)
    for core in range(N_CORES):
        b, half = core // 2, core % 2
        out[b, half * TOK:(half + 1) * TOK, :] = res.results[core]["out_sl"]
    return out


# revision 42
# speedup vs baseline: 1.0697x; 1.0477x over previous
"""Trainium2 Bass kernel for nn_DiTBlock (HGRN-attention DiT block).

Sharding: 8 cores = 4 batches x 2 half-sequences (1024 tokens each).
All matmuls are exact-integer bf16 matmuls (activations quantized to
int8-range integers in bf16; ternary weights quantized on host).

v6 structure:
  - per-token-tile pipelining (LN/quant/round per 128-token tile)
  - activation transposes via DMA-XBAR (bf16); only the f32 h-transpose
    uses the PE
  - AllGathers overlapped (adaln AG under the LN-stats sweep, the scan
    carry AG under interleaved wg matmuls)
  - exact fp32 C_MAGIC rounding; subtract step on ScalarE
  - scan outputs (ha/cam) spilled to DRAM per chunk, reloaded for the
    carry fix; h2 spilled pre-round as bf16
  - weight loads on the scalar/gpsimd HWDGE queues so they never queue
    behind compute-dependent DMAs
"""
import functools
import numpy as np
import ml_dtypes

import concourse.bass as bass
import concourse.bacc as bacc_mod
import concourse.mybir as mybir
import concourse.tile as tile
from concourse.masks import make_identity
from concourse.bass_utils import run_bass_kernel_spmd

BF16 = ml_dtypes.bfloat16
F32 = mybir.dt.float32
BF = mybir.dt.bfloat16
U32 = mybir.dt.uint32
AL = mybir.AluOpType
AF = mybir.ActivationFunctionType
AX = mybir.AxisListType

B, T, D = 4, 2048, 1024
TOK = 1024          # tokens per core
NH, HD = 16, 64
MLP = 4096
N_CORES = 8
CMAGIC = float(1.5 * 2 ** 23)
MAGIC_U32 = 0x5F3759DF


def _quant_w(w):
    invws = float(np.clip(np.abs(w).mean(dtype=np.float64), 1e-5, None))
    m = np.clip(np.round(w.astype(np.float64) / invws), -1, 1).astype(np.float32)
    return np.ascontiguousarray(m.astype(BF16)), np.float32(invws)


def _build(iw):
    nc = bacc_mod.Bacc("TRN2", target_bir_lowering=False)

    x_sl = nc.declare_dram_parameter("x_sl", [TOK, D], F32, isOutput=False)
    c_cols = nc.declare_dram_parameter("c_cols", [128, 8, B], F32, isOutput=False)
    adw_sl = nc.declare_dram_parameter("adw_sl", [D, 768], F32, isOutput=False)
    adb_sl = nc.declare_dram_parameter("adb_sl", [1, 768], F32, isOutput=False)
    mask8 = nc.declare_dram_parameter("mask8", [N_CORES, 1], F32, isOutput=False)
    bsel = nc.declare_dram_parameter("bsel", [1, B], F32, isOutput=False)
    bmask = nc.declare_dram_parameter("bmask", [B, 1], F32, isOutput=False)
    gnr = nc.declare_dram_parameter("gnr", [1, D], F32, isOutput=False)
    wiT = nc.declare_dram_parameter("wiT", [D, D], BF, isOutput=False)
    wfT = nc.declare_dram_parameter("wfT", [D, D], BF, isOutput=False)
    wgT = nc.declare_dram_parameter("wgT", [D, D], BF, isOutput=False)
    woT = nc.declare_dram_parameter("woT", [D, D], BF, isOutput=False)
    gwT = nc.declare_dram_parameter("gwT", [D, 2 * MLP], BF, isOutput=False)
    dwT = nc.declare_dram_parameter("dwT", [MLP, D], BF, isOutput=False)
    out_sl = nc.declare_dram_parameter("out_sl", [TOK, D], F32, isOutput=True)

    cc1_in = nc.dram_tensor("cc1_in", [B, 768], F32)
    cc1_out = nc.dram_tensor("cc1_out", [N_CORES * B, 768], F32,
                             addr_space="Shared")
    cc2_in = nc.dram_tensor("cc2_in", [D], F32)
    cc2_out = nc.dram_tensor("cc2_out", [N_CORES, D], F32, addr_space="Shared")

    RG = [list(range(N_CORES))]

    with tile.TileContext(nc) as tc:
        import contextlib
        es = contextlib.ExitStack()
        with es:
            cst = es.enter_context(tc.tile_pool(name="cst", bufs=1))
            ps = es.enter_context(tc.tile_pool(name="ps", bufs=4, space="PSUM"))
            psm = es.enter_context(tc.tile_pool(name="psm", bufs=1, space="PSUM"))
            pst = es.enter_context(tc.tile_pool(name="pst", bufs=2, space="PSUM"))
            dr = es.enter_context(tc.tile_pool(name="dr", bufs=1, space="DRAM"))

            def pmm():
                return ps.tile([128, 512], F32, tag="mm", name="mm")

            def newton_rsqrt(sb, x_ap, scale, bias, shape, tag, iters=3):
                """rsqrt(x*scale + bias) via bit-magic + Newton iters."""
                t = sb.tile(shape, F32, tag=tag + "_t", name=tag + "_t")
                nc.vector.tensor_scalar(out=t, in0=x_ap, scalar1=float(scale),
                                        scalar2=float(bias), op0=AL.mult,
                                        op1=AL.add)
                y = sb.tile(shape, F32, tag=tag + "_y", name=tag + "_y")
                sh = sb.tile(shape, F32, tag=tag + "_s", name=tag + "_s")
                nc.vector.tensor_scalar(out=sh[:].bitcast(U32),
                                        in0=t[:].bitcast(U32), scalar1=1,
                                        scalar2=None,
                                        op0=AL.logical_shift_right)
                mg = sb.tile(shape, F32, tag=tag + "_m", name=tag + "_m")
                nc.vector.memset(mg[:].bitcast(U32), MAGIC_U32)
                nc.vector.tensor_tensor(out=y[:].bitcast(U32),
                                        in0=mg[:].bitcast(U32),
                                        in1=sh[:].bitcast(U32), op=AL.subtract)
                e = sb.tile(shape, F32, tag=tag + "_e", name=tag + "_e")
                for _ in range(iters):
                    nc.vector.tensor_tensor(out=e, in0=y, in1=y, op=AL.mult)
                    nc.vector.tensor_tensor(out=e, in0=e, in1=t, op=AL.mult)
                    nc.vector.tensor_scalar(out=e, in0=e, scalar1=-0.5,
                                            scalar2=1.5, op0=AL.mult,
                                            op1=AL.add)
                    nc.vector.tensor_tensor(out=y, in0=y, in1=e, op=AL.mult)
                return y

            def quant_cols(sb, am_col, ss_col, dk, q_col, dq_col, dq_scale,
                           shape, tag):
                """q = 127/max(am,1e-5); dq = am*rsqrt(ss/dk+1e-8)*scale/127"""
                amc = sb.tile(shape, F32, tag=tag + "amc", name=tag + "amc")
                nc.vector.tensor_scalar(out=amc, in0=am_col, scalar1=1e-5,
                                        scalar2=None, op0=AL.max)
                rec = sb.tile(shape, F32, tag=tag + "rec", name=tag + "rec")
                nc.vector.reciprocal(out=rec, in_=amc)
                nc.vector.tensor_scalar(out=q_col, in0=rec, scalar1=127.0,
                                        scalar2=None, op0=AL.mult)
                rs = newton_rsqrt(sb, ss_col, 1.0 / dk, 1e-8, shape,
                                  tag + "rs", iters=2)
                nc.vector.tensor_tensor(out=dq_col, in0=amc, in1=rs,
                                        op=AL.mult)
                nc.vector.tensor_scalar(out=dq_col, in0=dq_col,
                                        scalar1=float(dq_scale) / 127.0,
                                        scalar2=None, op0=AL.mult)

            # ---------------- consts (whole-kernel lifetime) -------------
            identf = cst.tile([128, 128], F32)
            make_identity(nc, identf)
            ones_row = cst.tile([1, 128], F32)
            nc.vector.memset(ones_row, 1.0)
            negmagic = cst.tile([128, 1], F32)
            nc.vector.memset(negmagic, -CMAGIC)
            mask_sb = cst.tile([N_CORES, 1], F32)
            nc.sync.dma_start(out=mask_sb, in_=mask8[:, :])
            bsel_sb = cst.tile([1, B], F32)
            nc.sync.dma_start(out=bsel_sb, in_=bsel[:, :])
            bmask_sb = cst.tile([B, 1], F32)
            nc.sync.dma_start(out=bmask_sb, in_=bmask[:, :])

            def round_bf16(sb, src, q_col, tagp, bufs=3):
                """round(src*q) -> integer-valued bf16 tile via fp32 magic."""
                t2 = sb.tile([128, src.free_size()], F32, tag=tagp + "t2",
                             name=tagp + "t2")
                nc.vector.tensor_scalar(out=t2, in0=src, scalar1=q_col,
                                        scalar2=CMAGIC, op0=AL.mult,
                                        op1=AL.add)
                kq = sb.tile([128, src.free_size()], BF, tag=tagp + "kq",
                             name=tagp + "kq", bufs=bufs)
                nc.scalar.activation(out=kq, in_=t2, func=AF.Identity,
                                     bias=negmagic)
                return kq

            q127A = cst.tile([128, 8], F32); dqA = cst.tile([128, 8], F32)
            dqAg = cst.tile([128, 8], F32)
            q127O = cst.tile([128, 8], F32); dqOo = cst.tile([128, 8], F32)
            q127C = cst.tile([128, 8], F32); dqCg = cst.tile([128, 8], F32)
            q127D = cst.tile([128, 8], F32); dqDo = cst.tile([128, 8], F32)
            B_gn = cst.tile([128, D], F32)
            B_g1 = cst.tile([128, D], F32)
            B_sh2 = cst.tile([128, D], F32)
            B_sc2 = cst.tile([128, D], F32)
            B_g2 = cst.tile([128, D], F32)

            params_d = dr.tile([6 * D], F32, tag="params")
            ha_d = dr.tile([TOK, D], F32, tag="had")
            cam_d = dr.tile([TOK, D], BF, tag="camd")
            h2_d = dr.tile([TOK, MLP], BF, tag="h2d")

            # whole-kernel SBUF: xqT (-> x2qT) and xnew
            big = es.enter_context(tc.tile_pool(name="big", bufs=1))
            xqT = big.tile([128, 8, TOK], BF, tag="xq", bufs=1)

            def bcast_row(row_ap, dst, plus1=False):
                for ch in range(0, D, 512):
                    pb = pst.tile([128, 512], F32, tag="aux", name="aux")
                    nc.tensor.matmul(pb, ones_row, row_ap[:, ch:ch + 512],
                                     start=True, stop=True)
                    if plus1:
                        nc.scalar.activation(out=dst[:, ch:ch + 512], in_=pb,
                                             func=AF.Identity, bias=1.0)
                    else:
                        nc.scalar.copy(out=dst[:, ch:ch + 512], in_=pb)

            # =========== phase 0: adaln (+AG) and LN stats ===========
            with tc.tile_pool(name="p0", bufs=2) as p0:
                B_sh1 = p0.tile([128, D], F32, tag="Bsh1", bufs=1)
                B_sc1 = p0.tile([128, D], F32, tag="Bsc1", bufs=1)
                gnr_sb = p0.tile([1, D], F32, tag="gnr", bufs=1)
                nc.sync.dma_start(out=gnr_sb, in_=gnr[:, :])

                # adaln: this core computes 768 of the 6144 outputs
                c_sb = p0.tile([128, 8, B], F32, tag="csb", bufs=1)
                nc.sync.dma_start(out=c_sb, in_=c_cols[:, :, :])
                cs_sb = p0.tile([128, 8, B], F32, tag="cssb", bufs=1)
                nc.scalar.activation(out=cs_sb, in_=c_sb, func=AF.Silu)
                adb_sb = p0.tile([1, 768], F32, tag="adb", bufs=1)
                nc.sync.dma_start(out=adb_sb, in_=adb_sl[:, :])
                psA = psm.tile([B, 512], F32, tag="sm")
                psB = psm.tile([B, 256], F32, tag="sm2")
                for j in range(8):
                    adw_j = p0.tile([128, 768], F32, tag="adw")
                    nc.sync.dma_start(out=adw_j,
                                      in_=adw_sl[128 * j:128 * (j + 1), :])
                    nc.tensor.matmul(psA, cs_sb[:, j, :], adw_j[:, 0:512],
                                     start=(j == 0), stop=False)
                    nc.tensor.matmul(psB, cs_sb[:, j, :], adw_j[:, 512:768],
                                     start=(j == 0), stop=False)
                nc.tensor.matmul(psA, bsel_sb, adb_sb[:, 0:512],
                                 start=False, stop=True)
                nc.tensor.matmul(psB, bsel_sb, adb_sb[:, 512:768],
                                 start=False, stop=True)
                ad_sb = p0.tile([B, 768], F32, tag="adsb", bufs=1)
                nc.scalar.copy(out=ad_sb[:, 0:512], in_=psA)
                nc.scalar.copy(out=ad_sb[:, 512:768], in_=psB)
                nc.sync.dma_start(out=cc1_in[:, :], in_=ad_sb)
                nc.gpsimd.collective_compute(
                    "AllGather", AL.bypass, ins=[cc1_in[:]],
                    outs=[cc1_out[:]], replica_groups=RG)

                # LN stats pass (params-independent -> overlaps the AG)
                amA = p0.tile([128, 8], F32, tag="amA", bufs=1)
                ssA = p0.tile([128, 8], F32, tag="ssA", bufs=1)
                rstdA = p0.tile([128, 8], F32, tag="rstdA", bufs=1)
                nmrA = p0.tile([128, 8], F32, tag="nmrA", bufs=1)
                for i in range(8):
                    xt = p0.tile([128, D], F32, tag="xt", bufs=2)
                    nc.sync.dma_start(out=xt,
                                      in_=x_sl[128 * i:128 * (i + 1), :])
                    st = p0.tile([128, 2, 6], F32, tag="bst")
                    xr = xt.rearrange("p (s d) -> p s d", s=2)
                    for s2 in range(2):
                        nc.vector.bn_stats(out=st[:, s2, :], in_=xr[:, s2, :])
                    mv = p0.tile([128, 2], F32, tag="bmv")
                    nc.vector.bn_aggr(out=mv, in_=st)
                    rstdLN = newton_rsqrt(p0, mv[:, 1:2], 1.0, 1e-6,
                                          [128, 1], "rLN")
                    nc.vector.tensor_copy(out=rstdA[:, i:i + 1], in_=rstdLN)
                    nc.vector.tensor_tensor(out=nmrA[:, i:i + 1],
                                            in0=mv[:, 0:1], in1=rstdLN,
                                            op=AL.mult)
                nc.vector.tensor_scalar(out=nmrA, in0=nmrA, scalar1=-1.0,
                                        scalar2=None, op0=AL.mult)

                # select this batch's row per 768-block, stage through DRAM
                for r in range(8):
                    ag_r = p0.tile([B, 768], F32, tag="ag1")
                    nc.sync.dma_start(out=ag_r,
                                      in_=cc1_out[4 * r:4 * (r + 1), :])
                    pp1 = psm.tile([1, 512], F32, tag="sm")
                    pp2 = psm.tile([1, 256], F32, tag="sm2")
                    nc.tensor.matmul(pp1, bmask_sb, ag_r[:, 0:512],
                                     start=True, stop=True)
                    nc.tensor.matmul(pp2, bmask_sb, ag_r[:, 512:768],
                                     start=True, stop=True)
                    rb7 = p0.tile([1, 768], F32, tag="rb7", bufs=2)
                    nc.scalar.copy(out=rb7[:, 0:512], in_=pp1)
                    nc.scalar.copy(out=rb7[:, 512:768], in_=pp2)
                    nc.sync.dma_start(
                        out=params_d[768 * r:768 * (r + 1)].rearrange(
                            "(one c) -> one c", one=1),
                        in_=rb7)
                B_list = [(B_sh1, False), (B_sc1, True), (B_g1, False),
                          (B_sh2, False), (B_sc2, True), (B_g2, False)]
                for k, (dst, plus1) in enumerate(B_list):
                    rbk = p0.tile([1, D], F32, tag="rbk", bufs=2)
                    nc.sync.dma_start(
                        out=rbk,
                        in_=params_d[D * k:D * (k + 1)].rearrange(
                            "(one c) -> one c", one=1))
                    bcast_row(rbk, dst, plus1=plus1)
                bcast_row(gnr_sb, B_gn)

                # ===== phase 1: modulate + quant + round (per tile) =====
                for i in range(8):
                    xt = p0.tile([128, D], F32, tag="xt2", bufs=3)
                    nc.sync.dma_start(out=xt,
                                      in_=x_sl[128 * i:128 * (i + 1), :])
                    u = p0.tile([128, D], F32, tag="u")
                    nc.scalar.activation(out=u, in_=xt, func=AF.Identity,
                                         scale=rstdA[:, i:i + 1],
                                         bias=nmrA[:, i:i + 1])
                    ttm = p0.tile([128, D], F32, tag="ttm")
                    nc.vector.tensor_tensor(out=ttm, in0=u, in1=B_sc1,
                                            op=AL.mult)
                    moda = p0.tile([128, D], F32, tag="moda", bufs=3)
                    nc.vector.tensor_tensor(out=moda, in0=ttm, in1=B_sh1,
                                            op=AL.add)
                    nc.vector.tensor_reduce(out=amA[:, i:i + 1], in_=moda,
                                            axis=AX.X, op=AL.max,
                                            apply_absolute_value=True)
                    sqs = p0.tile([128, D], BF, tag="sqs")
                    nc.scalar.activation(out=sqs, in_=moda, func=AF.Square,
                                         accum_out=ssA[:, i:i + 1])
                    quant_cols(p0, amA[:, i:i + 1], ssA[:, i:i + 1], D,
                               q127A[:, i:i + 1], dqA[:, i:i + 1], 1.0,
                               [128, 1], "qa")
                    nc.vector.tensor_scalar(out=dqAg[:, i:i + 1],
                                            in0=dqA[:, i:i + 1],
                                            scalar1=float(iw["g"]),
                                            scalar2=None, op0=AL.mult)
                    kq = round_bf16(p0, moda, q127A[:, i:i + 1], "ra")
                    nc.sync.dma_start(out=xqT[:, :, 128 * i:128 * (i + 1)],
                                      in_=kq, transpose=True)

            # ====== phase 2: i/f + g matmuls + scan (chunked) ======
            pG = tc.tile_pool(name="pG", bufs=1)
            pgs = pG.__enter__()
            gs = pgs.tile([128, 8, D], F32, tag="gs", bufs=1)
            with tc.tile_pool(name="p2", bufs=2) as p2:
                # Sb_i / Sb_f: dq row broadcast over partitions, scaled by iw
                Sb_i = p2.tile([128, TOK], F32, tag="Sbi", bufs=1)
                Sb_f = p2.tile([128, TOK], F32, tag="Sbf", bufs=1)
                dqrow_sb = p2.tile([1, D], F32, tag="dqrow", bufs=1)
                for i8 in range(8):
                    ptr = psm.tile([1, 128], F32, tag="sm")
                    nc.tensor.transpose(ptr, dqA[:, i8:i8 + 1], identf)
                    nc.scalar.copy(out=dqrow_sb[:, 128 * i8:128 * (i8 + 1)],
                                   in_=ptr)
                oi = p2.tile([1, 128], F32, tag="oi", bufs=1)
                nc.vector.memset(oi, float(iw["i"]))
                of = p2.tile([1, 128], F32, tag="of", bufs=1)
                nc.vector.memset(of, float(iw["f"]))
                for ch in range(0, TOK, 512):
                    pb = pst.tile([128, 512], F32, tag="aux", name="aux")
                    nc.tensor.matmul(pb, oi, dqrow_sb[:, ch:ch + 512],
                                     start=True, stop=True)
                    nc.scalar.copy(out=Sb_i[:, ch:ch + 512], in_=pb)
                    pb2 = pst.tile([128, 512], F32, tag="aux", name="aux")
                    nc.tensor.matmul(pb2, of, dqrow_sb[:, ch:ch + 512],
                                     start=True, stop=True)
                    nc.scalar.copy(out=Sb_f[:, ch:ch + 512], in_=pb2)

                wf_sb = p2.tile([128, 8, D], BF, tag="wfsb", bufs=1)
                nc.scalar.dma_start(
                    out=wf_sb,
                    in_=wfT[:, :].rearrange("(a p) q -> p a q", p=128))
                wi_sb = p2.tile([128, 8, D], BF, tag="wisb", bufs=1)
                nc.scalar.dma_start(
                    out=wi_sb,
                    in_=wiT[:, :].rearrange("(a p) q -> p a q", p=128))
                wg_sb = p2.tile([128, 8, D], BF, tag="wgsb", bufs=1)
                nc.scalar.dma_start(
                    out=wg_sb,
                    in_=wgT[:, :].rearrange("(a p) q -> p a q", p=128))

                def g_mms(trange):
                    for t in trange:
                        for ck2 in range(0, D, 512):
                            pg = pmm()
                            for j in range(8):
                                nc.tensor.matmul(
                                    pg, xqT[:, j, 128 * t:128 * (t + 1)],
                                    wg_sb[:, j, ck2:ck2 + 512],
                                    start=(j == 0), stop=(j == 7))
                            scr = p2.tile([128, 512], F32, tag="gscr",
                                          bufs=2)
                            nc.scalar.activation(out=scr, in_=pg,
                                                 func=AF.Silu,
                                                 scale=dqAg[:, t:t + 1])
                            nc.vector.tensor_tensor(
                                out=gs[:, t, ck2:ck2 + 512], in0=scr,
                                in1=B_gn[:, ck2:ck2 + 512], op=AL.mult)

                ha_last = p2.tile([128, 8], F32, tag="halast", bufs=1)
                cam_last = p2.tile([128, 8], F32, tag="camlast", bufs=1)

                def scan_chain(ft, it, m, ck, cki):
                    sigf = p2.tile([128, 512], F32, tag="sigf", bufs=2)
                    nc.scalar.activation(out=sigf, in_=ft, func=AF.Sigmoid)
                    omf = p2.tile([128, 512], F32, tag="omf", bufs=2)
                    nc.scalar.activation(out=omf, in_=ft, func=AF.Sigmoid,
                                         scale=-1.0)
                    sili = p2.tile([128, 512], F32, tag="sili", bufs=2)
                    nc.scalar.activation(out=sili, in_=it, func=AF.Silu)
                    ifin = p2.tile([128, 512], F32, tag="ifin", bufs=2)
                    nc.vector.tensor_tensor(out=ifin, in0=sili, in1=omf,
                                            op=AL.mult)
                    ha_c = p2.tile([128, 512], F32, tag="hac", bufs=3)
                    init_h = 0.0 if cki == 0 else ha_last[:, m:m + 1]
                    nc.vector.tensor_tensor_scan(
                        ha_c, sigf, ifin, init_h, op0=AL.mult, op1=AL.add)
                    cam_c = p2.tile([128, 512], BF, tag="camc", bufs=3)
                    init_c = 1.0 if cki == 0 else cam_last[:, m:m + 1]
                    nc.vector.tensor_tensor_scan(
                        cam_c, sigf, sigf, init_c, op0=AL.mult, op1=AL.bypass)
                    nc.sync.dma_start(
                        out=ha_d[128 * m:128 * (m + 1), ck:ck + 512],
                        in_=ha_c)
                    nc.sync.dma_start(
                        out=cam_d[128 * m:128 * (m + 1), ck:ck + 512],
                        in_=cam_c)
                    if cki == 0:
                        nc.vector.tensor_copy(out=ha_last[:, m:m + 1],
                                              in_=ha_c[:, 511:512])
                        nc.vector.tensor_copy(out=cam_last[:, m:m + 1],
                                              in_=cam_c[:, 511:512])
                    else:
                        nc.sync.dma_start(
                            out=cc2_in[128 * m:128 * (m + 1)].rearrange(
                                "(p one) -> p one", one=1),
                            in_=ha_c[:, 511:512])

                pending = None
                for cki, ck in enumerate(range(0, TOK, 512)):
                    for m in range(8):
                        pf = pmm()
                        pi = pmm()
                        for j in range(8):
                            nc.tensor.matmul(
                                pf, wf_sb[:, j, 128 * m:128 * (m + 1)],
                                xqT[:, j, ck:ck + 512],
                                start=(j == 0), stop=(j == 7))
                        for j in range(8):
                            nc.tensor.matmul(
                                pi, wi_sb[:, j, 128 * m:128 * (m + 1)],
                                xqT[:, j, ck:ck + 512],
                                start=(j == 0), stop=(j == 7))
                        ft = p2.tile([128, 512], F32, tag="ftm", bufs=3)
                        nc.vector.tensor_tensor(out=ft, in0=pf,
                                                in1=Sb_f[:, ck:ck + 512],
                                                op=AL.mult)
                        it = p2.tile([128, 512], F32, tag="itm", bufs=3)
                        nc.vector.tensor_tensor(out=it, in0=pi,
                                                in1=Sb_i[:, ck:ck + 512],
                                                op=AL.mult)
                        if pending is not None:
                            scan_chain(*pending)
                        pending = (ft, it, m, ck, cki)
                    if cki == 1:
                        scan_chain(*pending)
                        pending = None
                    g_mms(range(0, 4) if cki == 0 else range(4, 8))
                nc.gpsimd.collective_compute(
                    "AllGather", AL.bypass, ins=[cc2_in[:]], outs=[cc2_out[:]],
                    replica_groups=RG)

            # =========== phase 3: carry fix + hT (PE transpose) ==========
            pO = tc.tile_pool(name="pO", bufs=1)
            pos_ = pO.__enter__()
            oqT = pos_.tile([128, 8, D], BF, tag="oqT", bufs=1)
            wo_sb = pos_.tile([128, 8, D], BF, tag="wosb", bufs=1)
            nc.scalar.dma_start(
                out=wo_sb,
                in_=woT[:, :].rearrange("(a p) q -> p a q", p=128))
            pH3 = tc.tile_pool(name="pH3", bufs=1)
            ph3_ = pH3.__enter__()
            hT = ph3_.tile([128, 8, D], F32, tag="hT", bufs=1)
            with tc.tile_pool(name="p3", bufs=2) as p3:
                ag2 = p3.tile([N_CORES, D], F32, tag="ag2", bufs=1)
                nc.sync.dma_start(out=ag2, in_=cc2_out[:, :])
                for m in range(8):
                    pc = psm.tile([128, 1], F32, tag="sm")
                    nc.tensor.matmul(pc, ag2[:, 128 * m:128 * (m + 1)],
                                     mask_sb, start=True, stop=True)
                    carry = p3.tile([128, 1], F32, tag="carry")
                    nc.scalar.copy(out=carry, in_=pc)
                    har = p3.tile([128, TOK], F32, tag="har", bufs=3)
                    nc.sync.dma_start(out=har,
                                      in_=ha_d[128 * m:128 * (m + 1), :])
                    camr = p3.tile([128, TOK], BF, tag="camr", bufs=3)
                    nc.sync.dma_start(out=camr,
                                      in_=cam_d[128 * m:128 * (m + 1), :])
                    hfix = p3.tile([128, TOK], F32, tag="hfix", bufs=2)
                    nc.vector.scalar_tensor_tensor(out=hfix, in0=camr,
                                                   scalar=carry, in1=har,
                                                   op0=AL.mult, op1=AL.add)
                    for g4 in range(0, 8, 4):
                        tp = pst.tile([128, 512], F32, tag="aux")
                        for jj in range(4):
                            t_i = g4 + jj
                            nc.tensor.transpose(
                                tp[:, 128 * jj:128 * (jj + 1)],
                                hfix[:, 128 * t_i:128 * (t_i + 1)], identf)
                        for jj in range(4):
                            t_i = g4 + jj
                            if jj % 2 == 0:
                                nc.scalar.copy(
                                    out=hT[:, t_i, 128 * m:128 * (m + 1)],
                                    in_=tp[:, 128 * jj:128 * (jj + 1)])
                            else:
                                nc.vector.tensor_copy(
                                    out=hT[:, t_i, 128 * m:128 * (m + 1)],
                                    in_=tp[:, 128 * jj:128 * (jj + 1)])

            # =========== phase 4: gnorm-swish-gate + o quant ===========
            with tc.tile_pool(name="p4", bufs=2) as p4:
                amO = p4.tile([128, 8], F32, tag="amO", bufs=1)
                ssO = p4.tile([128, 8], F32, tag="ssO", bufs=1)
                for t in range(8):
                    sq = p4.tile([128, D], F32, tag="sq")
                    nc.scalar.activation(out=sq, in_=hT[:, t, :],
                                         func=AF.Square)
                    msh = p4.tile([128, 16], F32, tag="msh")
                    nc.vector.tensor_reduce(
                        out=msh,
                        in_=sq.rearrange("p (h d) -> p h d", h=NH),
                        axis=AX.X, op=AL.add)
                    rstdH = newton_rsqrt(p4, msh, 1.0 / HD, 1e-5, [128, 16],
                                         "rH")
                    hn = p4.tile([128, D], F32, tag="hn")
                    rb = bass.AP(tensor=rstdH.tensor, offset=rstdH.offset,
                                 ap=[rstdH.ap[0], [1, NH], [0, HD]])
                    nc.vector.tensor_tensor(
                        out=hn.rearrange("p (h d) -> p h d", h=NH),
                        in0=hT[:, t, :].rearrange("p (h d) -> p h d", h=NH),
                        in1=rb, op=AL.mult)
                    oa = p4.tile([128, D], F32, tag="oa", bufs=2)
                    nc.vector.tensor_tensor(out=oa, in0=hn, in1=gs[:, t, :],
                                            op=AL.mult)
                    nc.vector.tensor_reduce(out=amO[:, t:t + 1], in_=oa,
                                            axis=AX.X, op=AL.max,
                                            apply_absolute_value=True)
                    sqo = p4.tile([128, D], BF, tag="sqo", bufs=1)
                    nc.scalar.activation(out=sqo, in_=oa, func=AF.Square,
                                         accum_out=ssO[:, t:t + 1])
                    quant_cols(p4, amO[:, t:t + 1], ssO[:, t:t + 1], D,
                               q127O[:, t:t + 1], dqOo[:, t:t + 1], iw["o"],
                               [128, 1], "qo")
                    kq = round_bf16(p4, oa, q127O[:, t:t + 1], "ro")
                    nc.sync.dma_start(out=oqT[:, :, 128 * t:128 * (t + 1)],
                                      in_=kq, transpose=True)
            pH3.__exit__(None, None, None)  # free hT

            # ====== phase 5: wo matmul + residual + LN2 + quant ======
            xnew = big.tile([128, 8, D], F32, tag="xnew", bufs=1)
            x2qT = big.tile([128, 8, TOK], BF, tag="xq", bufs=1)
            with tc.tile_pool(name="p5", bufs=2) as p5:
                amC = p5.tile([128, 8], F32, tag="amC", bufs=1)
                ssC = p5.tile([128, 8], F32, tag="ssC", bufs=1)
                for t in range(8):
                    xr2 = p5.tile([128, D], F32, tag="xr2", bufs=3)
                    nc.sync.dma_start(out=xr2,
                                      in_=x_sl[128 * t:128 * (t + 1), :])
                    xn = xnew[:, t, :]
                    for ck in range(0, D, 512):
                        pw = pmm()
                        for j in range(8):
                            nc.tensor.matmul(
                                pw, oqT[:, j, 128 * t:128 * (t + 1)],
                                wo_sb[:, j, ck:ck + 512],
                                start=(j == 0), stop=(j == 7))
                        ug = p5.tile([128, 512], F32, tag="ug")
                        nc.vector.scalar_tensor_tensor(
                            out=ug, in0=pw, scalar=dqOo[:, t:t + 1],
                            in1=B_g1[:, ck:ck + 512],
                            op0=AL.mult, op1=AL.mult)
                        nc.vector.tensor_tensor(out=xn[:, ck:ck + 512],
                                                in0=ug,
                                                in1=xr2[:, ck:ck + 512],
                                                op=AL.add)
                    st = p5.tile([128, 2, 6], F32, tag="bst2")
                    xrr = xn.rearrange("p (s d) -> p s d", s=2)
                    for s2 in range(2):
                        nc.vector.bn_stats(out=st[:, s2, :], in_=xrr[:, s2, :])
                    mv = p5.tile([128, 2], F32, tag="bmv2")
                    nc.vector.bn_aggr(out=mv, in_=st)
                    rstdC = newton_rsqrt(p5, mv[:, 1:2], 1.0, 1e-6, [128, 1],
                                         "rC")
                    nmrC = p5.tile([128, 1], F32, tag="nmrC")
                    nc.vector.tensor_tensor(out=nmrC, in0=mv[:, 0:1],
                                            in1=rstdC, op=AL.mult)
                    nc.vector.tensor_scalar(out=nmrC, in0=nmrC, scalar1=-1.0,
                                            scalar2=None, op0=AL.mult)
                    u2 = p5.tile([128, D], F32, tag="u2")
                    nc.scalar.activation(out=u2, in_=xn, func=AF.Identity,
                                         scale=rstdC, bias=nmrC)
                    tt2 = p5.tile([128, D], F32, tag="tt2")
                    nc.vector.tensor_tensor(out=tt2, in0=u2, in1=B_sc2,
                                            op=AL.mult)
                    mod2 = p5.tile([128, D], F32, tag="mod2", bufs=2)
                    nc.vector.tensor_tensor(out=mod2, in0=tt2, in1=B_sh2,
                                            op=AL.add)
                    nc.vector.tensor_reduce(out=amC[:, t:t + 1], in_=mod2,
                                            axis=AX.X, op=AL.max,
                                            apply_absolute_value=True)
                    sqc = p5.tile([128, D], BF, tag="sqc")
                    nc.scalar.activation(out=sqc, in_=mod2, func=AF.Square,
                                         accum_out=ssC[:, t:t + 1])
                    quant_cols(p5, amC[:, t:t + 1], ssC[:, t:t + 1], D,
                               q127C[:, t:t + 1], dqCg[:, t:t + 1], iw["gate"],
                               [128, 1], "qc")
                    kq = round_bf16(p5, mod2, q127C[:, t:t + 1], "rc")
                    nc.sync.dma_start(out=x2qT[:, :, 128 * t:128 * (t + 1)],
                                      in_=kq, transpose=True)
            pO.__exit__(None, None, None)   # free oqT/wo
            pG.__exit__(None, None, None)   # free gs

            # =========== phase 6: MLP gate matmuls -> h2 (bf16) ==========
            pDW = tc.tile_pool(name="pDW", bufs=1)
            pdw = pDW.__enter__()
            dw_sb = pdw.tile([128, 32, D], BF, tag="dwsb", bufs=1)
            nc.gpsimd.dma_start(
                out=dw_sb,
                in_=dwT[:, :].rearrange("(a p) q -> p a q", p=128))
            with tc.tile_pool(name="p6", bufs=2) as p6:
                amDg = p6.tile([128, 8, 8], F32, tag="amDg", bufs=1)
                ssDg = p6.tile([128, 8, 8], F32, tag="ssDg", bufs=1)
                for g in range(8):
                    gw_g = p6.tile([128, 8, 1024], BF, tag="gwg")
                    nc.gpsimd.dma_start(
                        out=gw_g[:, :, 0:512],
                        in_=gwT[:, 512 * g:512 * (g + 1)].rearrange(
                            "(a p) q -> p a q", p=128))
                    nc.gpsimd.dma_start(
                        out=gw_g[:, :, 512:1024],
                        in_=gwT[:, MLP + 512 * g:MLP + 512 * (g + 1)].rearrange(
                            "(a p) q -> p a q", p=128))
                    for t in range(8):
                        pg = pmm()
                        py = pmm()
                        for j in range(8):
                            nc.tensor.matmul(
                                pg, x2qT[:, j, 128 * t:128 * (t + 1)],
                                gw_g[:, j, 0:512],
                                start=(j == 0), stop=(j == 7))
                        for j in range(8):
                            nc.tensor.matmul(
                                py, x2qT[:, j, 128 * t:128 * (t + 1)],
                                gw_g[:, j, 512:1024],
                                start=(j == 0), stop=(j == 7))
                        sil = p6.tile([128, 512], F32, tag="sil")
                        nc.scalar.activation(out=sil, in_=pg, func=AF.Silu,
                                             scale=dqCg[:, t:t + 1])
                        h2c = p6.tile([128, 512], BF, tag="h2c", bufs=3)
                        nc.vector.scalar_tensor_tensor(
                            out=h2c, in0=py, scalar=dqCg[:, t:t + 1],
                            in1=sil, op0=AL.mult, op1=AL.mult)
                        nc.sync.dma_start(
                            out=h2_d[128 * t:128 * (t + 1),
                                     512 * g:512 * (g + 1)],
                            in_=h2c)
                        nc.vector.tensor_reduce(out=amDg[:, t, g:g + 1],
                                                in_=h2c, axis=AX.X, op=AL.max,
                                                apply_absolute_value=True)
                        sqd = p6.tile([128, 512], BF, tag="sqd")
                        nc.scalar.activation(out=sqd, in_=h2c, func=AF.Square,
                                             accum_out=ssDg[:, t, g:g + 1])
                amD = p6.tile([128, 8], F32, tag="amD", bufs=1)
                ssD = p6.tile([128, 8], F32, tag="ssD", bufs=1)
                nc.vector.tensor_reduce(out=amD, in_=amDg, axis=AX.X,
                                        op=AL.max)
                nc.vector.tensor_reduce(out=ssD, in_=ssDg, axis=AX.X,
                                        op=AL.add)
                quant_cols(p6, amD, ssD, MLP, q127D, dqDo, iw["down"],
                           [128, 8], "qd")

            # ====== phase 7: round h2 + down matmuls + final residual ====
            with tc.tile_pool(name="p7", bufs=2) as p7:
                for t in range(8):
                    h2t = p7.tile([128, 32, 128], BF, tag="h2t")
                    h2r = p7.tile([128, MLP], BF, tag="h2r", bufs=2)
                    nc.scalar.dma_start(out=h2r,
                                        in_=h2_d[128 * t:128 * (t + 1), :])
                    for q in range(4):
                        kqd = round_bf16(p7, h2r[:, 1024 * q:1024 * (q + 1)],
                                         q127D[:, t:t + 1], "rd", bufs=2)
                        nc.sync.dma_start(out=h2t[:, 8 * q:8 * (q + 1), :],
                                          in_=kqd, transpose=True)
                    outt = p7.tile([128, D], F32, tag="outt")
                    for ck in range(0, D, 512):
                        pdn = pmm()
                        for j2 in range(32):
                            nc.tensor.matmul(pdn, h2t[:, j2, :],
                                             dw_sb[:, j2, ck:ck + 512],
                                             start=(j2 == 0), stop=(j2 == 31))
                        uv = p7.tile([128, 512], F32, tag="uv")
                        nc.vector.scalar_tensor_tensor(
                            out=uv, in0=pdn, scalar=dqDo[:, t:t + 1],
                            in1=B_g2[:, ck:ck + 512],
                            op0=AL.mult, op1=AL.mult)
                        nc.vector.tensor_tensor(out=outt[:, ck:ck + 512],
                                                in0=uv,
                                                in1=xnew[:, t, ck:ck + 512],
                                                op=AL.add)
                    nc.scalar.dma_start(out=out_sl[128 * t:128 * (t + 1), :],
                                        in_=outt)
            pDW.__exit__(None, None, None)

    nc.finalize()
    return nc


@functools.lru_cache(maxsize=2)
def _build_cached(iw_items):
    return _build(dict(iw_items))


def kernel(x, c, adaln_w, adaln_b, wi, wf, wg, gnorm_w, wo, gate_w, down_w):
    x = np.ascontiguousarray(np.asarray(x, dtype=np.float32))
    c = np.ascontiguousarray(np.asarray(c, dtype=np.float32))
    adaln_w = np.asarray(adaln_w, dtype=np.float32)
    adaln_b = np.asarray(adaln_b, dtype=np.float32)
    gnorm_w = np.asarray(gnorm_w, dtype=np.float32)

    mi, iwi = _quant_w(np.asarray(wi, dtype=np.float32))
    mf, iwf = _quant_w(np.asarray(wf, dtype=np.float32))
    mg, iwg = _quant_w(np.asarray(wg, dtype=np.float32))
    mo, iwo = _quant_w(np.asarray(wo, dtype=np.float32))
    mgate, iwgate = _quant_w(np.asarray(gate_w, dtype=np.float32))
    mdown, iwdown = _quant_w(np.asarray(down_w, dtype=np.float32))

    iw = {"i": float(iwi), "f": float(iwf), "g": float(iwg), "o": float(iwo),
          "gate": float(iwgate), "down": float(iwdown)}
    nc = _build_cached(tuple(sorted(iw.items())))

    wiT_h = np.ascontiguousarray(mi.T)
    wfT_h = np.ascontiguousarray(mf.T)
    wgT_h = np.ascontiguousarray(mg.T)
    woT_h = np.ascontiguousarray(mo.T)
    gwT_h = np.ascontiguousarray(mgate.T)
    dwT_h = np.ascontiguousarray(mdown.T)
    adwT = np.ascontiguousarray(adaln_w.T)          # [D, 6D]
    gnr_h = np.ascontiguousarray(np.tile(gnorm_w, NH)[None, :])
    c_cols_h = np.ascontiguousarray(
        c.T.reshape(8, 128, B).transpose(1, 0, 2))   # [128, 8, B]

    in_maps = []
    for core in range(N_CORES):
        b, half = core // 2, core % 2
        mask = np.zeros((N_CORES, 1), np.float32)
        if half == 1:
            mask[core - 1, 0] = 1.0
        bm = np.zeros((B, 1), np.float32)
        bm[b, 0] = 1.0
        in_maps.append({
            "x_sl": np.ascontiguousarray(x[b, half * TOK:(half + 1) * TOK, :]),
            "c_cols": c_cols_h,
            "adw_sl": np.ascontiguousarray(adwT[:, 768 * core:768 * (core + 1)]),
            "adb_sl": np.ascontiguousarray(
                adaln_b[None, 768 * core:768 * (core + 1)]),
            "mask8": mask,
            "bsel": np.ones((1, B), np.float32),
            "bmask": bm,
            "gnr": gnr_h,
            "wiT": wiT_h, "wfT": wfT_h, "wgT": wgT_h, "woT": woT_h,
            "gwT": gwT_h, "dwT": dwT_h,
        })

    res = run_bass_kernel_spmd(nc, in_maps, core_ids=list(range(N_CORES)))
    out = np.zeros((B, T, D), np.float32)
    for core in range(N_CORES):
        b, half = core // 2, core % 2
        out[b, half * TOK:(half + 1) * TOK, :] = res.results[core]["out_sl"]
    return out
